# revision 1
# baseline (speedup 1.0000x reference)
"""BEV cross-attention kernel for Trainium2, 8-core SPMD.

Shard: core c handles (batch b=c//4, head m=c%4). Full attention for one
(b, head): per-camera QK^T (Q=1024, K=6*1680), softmax over 10080 keys,
P@V, then partial output projection; AllReduce over the 4 cores of each
batch merges heads; final skip+LN+MLP+LN computed redundantly per group.

Layout strategy: feature-major ("S^T") attention — scores [k_chunk=128p,
q=1024f] so softmax exp runs on ScalarE with per-partition scale=rstd_k
(K LayerNorm) and bias=ln(rstd_v) (V LayerNorm folded through exp).
LayerNorm means are folded into projection weights host-side; the softmax
denominator rides the PV matmul as an all-ones column of V. No max
subtraction (logits are small by construction: |logit| < ~2).
"""
import numpy as np

import concourse.bass as bass
import concourse.bass_isa as bass_isa
import concourse.mybir as mybir
import concourse.tile as tile
from concourse.bass_utils import run_bass_kernel_spmd

F32 = mybir.dt.float32
F32R = mybir.dt.float32r

HEADS, DH, D = 4, 32, 128
B, NCAM = 2, 6
Q = 32 * 32            # 1024 BEV queries
KC = 28 * 60           # 1680 keys per camera
NKCH = (KC + 127) // 128   # 14 k-chunks per camera (last has 16 rows)
KFULL = KC // 128          # 13 full chunks
KTAIL = KC - KFULL * 128   # 16
N_CORES = 8
EPS = 1e-5
SCALE = DH ** -0.5

_cached = {}


# ---------------------------------------------------------------------------
# walrus compat: this container's walrus rejects instructions carrying more
# than one semaphore wait; move excess waits onto same-engine NoOps.
_COMPUTE_ENGINES = None
_nopctr = [0]


def _split_sync_waits(nc, limit=1):
    global _COMPUTE_ENGINES
    if _COMPUTE_ENGINES is None:
        _COMPUTE_ENGINES = {
            mybir.EngineType.PE, mybir.EngineType.Activation,
            mybir.EngineType.Pool, mybir.EngineType.DVE, mybir.EngineType.SP,
        }
    for f in nc.m.functions:
        for bb in f.blocks:
            out, changed = [], False
            for inst in bb.instructions:
                si = inst.sync_info
                if (si is not None and len(si.on_wait) > limit
                        and inst.engine in _COMPUTE_ENGINES):
                    waits = list(si.on_wait)
                    n_extra = len(waits) - limit
                    for i in range(0, n_extra, limit):
                        nop = mybir.InstNoOp(name=f"wait-split-{_nopctr[0]}")
                        _nopctr[0] += 1
                        nop.engine = inst.engine
                        nop.sync_info = mybir.SyncInfo(
                            on_wait=waits[i:min(i + limit, n_extra)], on_update=[])
                        out.append(nop)
                    si.on_wait = waits[n_extra:]
                    changed = True
                out.append(inst)
            if changed:
                bb.instructions = out


# ---------------------------------------------------------------------------
def _build_program(split=True, collective=True, n_dev=N_CORES):
    nc = bass.Bass("TRN2", target_bir_lowering=False, debug=False,
                   num_devices=n_dev)

    def din(name, shape, dt=F32R):
        return nc.dram_tensor(name, shape, dt, kind="ExternalInput").ap()

    xq = din("xq", [NCAM, D, Q])
    xk = din("xk", [NCAM, D, KC])
    xv = din("xv", [NCAM, D, KC])
    wq_ext = din("wq_ext", [D, 33])      # [s*Wq'' | ones/128]
    wk_ext = din("wk_ext", [D, 33])      # [Wk'' | ones/128]
    wv_ext = din("wv_ext", [D, 34])      # [Wv'' | zeros | ones/128]
    wbq = din("wbq", [32, 1], F32)       # s * Wq_m^T @ bq_ln
    wbv = din("wbv", [33, 1], F32)       # [Wv_m^T @ bv_ln | 0]
    wp = din("wp", [32, D])              # Wp head slice (lhsT)
    bp = din("bp", [D, 1], F32)
    skipb = din("skipb", [D, Q], F32)
    w1 = din("w1", [D, 256])
    b1 = din("b1", [2, D, 1], F32)
    w2 = din("w2", [D, 2, D])            # [ff128, half, dout]
    b2 = din("b2", [D, 1], F32)
    pre_g = din("pre_g", [D, 1], F32)
    pre_b = din("pre_b", [D, 1], F32)
    post_g = din("post_g", [D, 1], F32)
    post_b = din("post_b", [D, 1], F32)
    onesv = din("onesv", [1, D])

    out = nc.dram_tensor("out", [D, Q], F32, kind="ExternalOutput").ap()


    EXP = mybir.ActivationFunctionType.Exp
    LN_ = mybir.ActivationFunctionType.Ln
    SQRT = mybir.ActivationFunctionType.Sqrt
    GELU = mybir.ActivationFunctionType.Gelu

    with tile.TileContext(nc) as tc:
        with tc.tile_pool(name="consts", bufs=1) as consts, \
             tc.tile_pool(name="loads", bufs=2) as loads, \
             tc.tile_pool(name="sq", bufs=1) as sqp, \
             tc.tile_pool(name="rows", bufs=3) as rows, \
             tc.tile_pool(name="sml", bufs=4) as sml, \
             tc.tile_pool(name="keep", bufs=1) as keep, \
             tc.tile_pool(name="ee", bufs=3) as eep, \
             tc.tile_pool(name="fin", bufs=1) as finp, \
             tc.tile_pool(name="dramp", bufs=6, space="DRAM") as dramp:

            def row_split(row2d, t_f, width, nm, pool, dt=F32):
                """[1, N] SBUF row -> [128, width] token-major tile, via a
                DRAM bounce (partition<->free reshape is not one DMA)."""
                n_el = row2d.shape[1]
                dsc = dramp.tile([n_el], dt, name=nm + "_d", tag="dsc")
                nc.sync.dma_start(out=dsc, in_=row2d)
                t = pool.tile([128, width], dt, name=nm, tag=nm)
                full = n_el // 128
                nc.sync.dma_start(
                    out=t[:, 0:full],
                    in_=dsc[0:full * 128].rearrange("(c t) -> t c", t=128))
                tail = n_el - full * 128
                if tail:
                    nc.vector.memset(t[:, full:full + 1], 0.0)
                    nc.sync.dma_start(
                        out=t[0:tail, full:full + 1],
                        in_=dsc[full * 128:].rearrange("(c t) -> t c", t=tail))
                return t

            def tm_join(tm_tile, n_el, nm, dt):
                """[128, c] token-major tile -> [1, n_el] SBUF row via DRAM
                bounce."""
                dsc = dramp.tile([n_el], dt, name=nm + "_d", tag="dsc")
                nc.sync.dma_start(
                    out=dsc.rearrange("(c t) -> t c", t=128), in_=tm_tile)
                row = rows.tile([1, n_el], dt, name=nm, tag="row")
                nc.sync.dma_start(out=row, in_=dsc)
                return row

            # ---- constants ----
            wq_t = consts.tile([D, 33], F32R, name="wq_t")
            nc.sync.dma_start(out=wq_t, in_=wq_ext)
            wk_t = consts.tile([D, 33], F32R, name="wk_t")
            nc.sync.dma_start(out=wk_t, in_=wk_ext)
            wv_t = consts.tile([D, 34], F32R, name="wv_t")
            nc.sync.dma_start(out=wv_t, in_=wv_ext)
            wbq_t = consts.tile([32, 1], F32, name="wbq_t")
            nc.sync.dma_start(out=wbq_t, in_=wbq)
            wbv_t = consts.tile([33, 1], F32, name="wbv_t")
            nc.sync.dma_start(out=wbv_t, in_=wbv)
            wp_t = consts.tile([32, D], F32R, name="wp_t")
            nc.sync.dma_start(out=wp_t, in_=wp)
            onesbc = consts.tile([1, D], F32R, name="onesbc")
            nc.sync.dma_start(out=onesbc, in_=onesv)
            eps_t = consts.tile([D, 1], F32, name="eps_t")
            nc.vector.memset(eps_t, EPS)

            # ---- per-camera projections + stats ----
            qhT = keep.tile([33, NCAM, Q], F32R, name="qhT")
            khT = keep.tile([33, NCAM, KC], F32R, name="khT")
            vhE = keep.tile([D, NCAM, NKCH, 34], mybir.dt.bfloat16, name="vhE")
            rstdk = keep.tile([D, NCAM, NKCH], F32, name="rstdk")
            lnrv = keep.tile([D, NCAM, NKCH], F32, name="lnrv")

            ph1 = tc.tile_pool(name="proj", bufs=1, space="PSUM")
            projp = ph1.__enter__()
            ph1b = tc.tile_pool(name="stat", bufs=1, space="PSUM")
            statp = ph1b.__enter__()
            for n in range(NCAM):
                xq_t = loads.tile([D, Q], F32R, name="xq_t", tag="xq_t")
                nc.sync.dma_start(out=xq_t, in_=xq[n])
                xk_t = loads.tile([D, KC], F32R, name="xk_t", tag="xk_t")
                nc.sync.dma_start(out=xk_t, in_=xk[n])
                xv_t = loads.tile([D, KC], F32R, name="xv_t", tag="xv_t")
                nc.sync.dma_start(out=xv_t, in_=xv[n])

                x2q = sqp.tile([D, Q], F32R, name="x2q", tag="x2q")
                nc.vector.tensor_mul(out=x2q, in0=xq_t, in1=xq_t)
                x2k = sqp.tile([D, KC], F32R, name="x2k", tag="x2k")
                nc.vector.tensor_mul(out=x2k, in0=xk_t, in1=xk_t)
                x2v = sqp.tile([D, KC], F32R, name="x2v", tag="x2v")
                nc.vector.tensor_mul(out=x2v, in0=xv_t, in1=xv_t)

                # Q/K projections (feature-major), col 32 = mean
                qp_ps = projp.tile([33, Q], F32, name="qp_ps", tag="qp_ps")
                for h in range(2):
                    nc.tensor.matmul(qp_ps[:, h * 512:(h + 1) * 512],
                                     lhsT=wq_t, rhs=xq_t[:, h * 512:(h + 1) * 512],
                                     start=True, stop=True)
                for hh in range(2):
                    kp_ps = projp.tile([33, 2, 512], F32, name="kp_ps",
                                       tag="kp_ps")
                    for h2 in range(2):
                        h = hh * 2 + h2
                        nc.tensor.matmul(
                            kp_ps[:, h2, 0:420], lhsT=wk_t,
                            rhs=xk_t[:, h * 420:(h + 1) * 420],
                            start=True, stop=True)
                    nc.vector.tensor_copy(
                        out=khT[:, n, hh * 840:(hh + 1) * 840].rearrange(
                            "p (h c) -> p h c", h=2),
                        in_=kp_ps[:, :, 0:420])
                # V projection (token-major) col 33 = mean
                vp_ps = projp.tile([D, NKCH, 34], F32, name="vp_ps", tag="vp_ps")
                for c in range(NKCH):
                    cw = 128 if c < KFULL else KTAIL
                    nc.tensor.matmul(vp_ps[0:cw, c, :],
                                     lhsT=xv_t[:, c * 128:c * 128 + cw],
                                     rhs=wv_t, start=True, stop=True)

                # sum-of-squares rows via GpSimd cross-partition reduce (SBUF)
                ssq = rows.tile([1, Q], F32, name="ssq", tag="row")
                nc.gpsimd.tensor_reduce(out=ssq, in_=x2q,
                                        axis=mybir.AxisListType.C,
                                        op=mybir.AluOpType.add)
                ssk = rows.tile([1, KC], F32, name="ssk", tag="row")
                nc.gpsimd.tensor_reduce(out=ssk, in_=x2k,
                                        axis=mybir.AxisListType.C,
                                        op=mybir.AluOpType.add)
                ssv = rows.tile([1, KC], F32, name="ssv", tag="row")
                nc.gpsimd.tensor_reduce(out=ssv, in_=x2v,
                                        axis=mybir.AxisListType.C,
                                        op=mybir.AluOpType.add)

                # ---- evacuate projections to SBUF ----
                qraw = sqp.tile([33, Q], F32, name="qraw", tag="qraw")
                nc.vector.tensor_copy(out=qraw, in_=qp_ps)
                nc.vector.tensor_copy(out=vhE[:, n, 0:KFULL, :],
                                      in_=vp_ps[:, 0:KFULL, :])
                nc.vector.memset(vhE[:, n, KFULL, :], 0.0)
                nc.vector.tensor_copy(out=vhE[0:KTAIL, n, KFULL, :],
                                      in_=vp_ps[0:KTAIL, KFULL, :])
                # ones column for softmax denominator
                nc.vector.memset(vhE[:, n, :, 32], 1.0)

                # ---- token-major stats via reshape DMAs ----
                muq = row_split(qraw[32:33, :], 128, 8, "muq", sml)
                msqq = row_split(ssq, 128, 8, "msqq", sml)
                muk = row_split(khT[32:33, n, :].bitcast(F32), 128, NKCH,
                                "muk", sml)
                msqk = row_split(ssk, 128, NKCH, "msqk", sml)
                msqv = row_split(ssv, 128, NKCH, "msqv", sml)

                # ---- rstd computation (token-major, batched) ----
                def make_rstd(mu_ap, msq_ap, out_ap, width, log=False):
                    v = sml.tile([128, width], F32, name="v_rstd", tag="v_rstd")
                    nc.vector.tensor_mul(out=v, in0=mu_ap, in1=mu_ap)
                    v2 = sml.tile([128, width], F32, name="v2_rstd",
                                  tag="v2_rstd")
                    nc.vector.tensor_scalar_mul(out=v2, in0=msq_ap,
                                                scalar1=1.0 / 128.0)
                    nc.vector.tensor_sub(out=v, in0=v2, in1=v)
                    if log:
                        # ln(rstd) = -0.5*ln(var+eps)
                        nc.scalar.activation(out=v, in_=v, func=LN_,
                                             bias=eps_t, scale=1.0)
                        nc.vector.tensor_scalar_mul(out=out_ap, in0=v,
                                                    scalar1=-0.5)
                    else:
                        # rstd = exp(-0.5*ln(var+eps)): keeps ScalarE on the
                        # natural_log_exp table set (no sqrt-table reloads)
                        nc.scalar.activation(out=v, in_=v, func=LN_,
                                             bias=eps_t, scale=1.0)
                        if out_ap.dtype == F32R:
                            vexp = sml.tile([128, width], F32, name="vexp",
                                            tag="vexp")
                            nc.scalar.activation(out=vexp, in_=v, func=EXP,
                                                 bias=0.0, scale=-0.5)
                            nc.vector.tensor_copy(out=out_ap, in_=vexp)
                        else:
                            nc.scalar.activation(out=out_ap, in_=v, func=EXP,
                                                 bias=0.0, scale=-0.5)

                rstdq = sml.tile([128, 8], F32R, name="rstdq", tag="rstdq")
                make_rstd(muq, msqq, rstdq, 8)
                make_rstd(muk, msqk, rstdk[:, n, :], NKCH)
                # V mean lives token-major in vhE col 33
                muv = sml.tile([128, NKCH], F32, name="muv", tag="muv")
                nc.vector.tensor_copy(out=muv, in_=vhE[:, n, :, 33])
                make_rstd(muv, msqv, lnrv[:, n, :], NKCH, log=True)

                # ---- finalize qhT: rstd_q broadcast & apply ----
                rq_row = tm_join(rstdq, Q, "rq_row", F32R)
                rq_bc = statp.tile([32, Q], F32, name="rq_bc", tag="rq_bc")
                for h in range(2):
                    nc.tensor.matmul(rq_bc[:, h * 512:(h + 1) * 512],
                                     lhsT=onesbc[:, 0:32],
                                     rhs=rq_row[:, h * 512:(h + 1) * 512],
                                     start=True, stop=True)
                nc.vector.tensor_mul(out=qhT[0:32, n, :], in0=qraw[0:32, :],
                                     in1=rq_bc)
                nc.vector.tensor_scalar_add(out=qhT[0:32, n, :],
                                            in0=qhT[0:32, n, :], scalar1=wbq_t)

            ph1b.__exit__(None, None, None)
            ph1.__exit__(None, None, None)

            # ---- attention ----
            ph2 = tc.tile_pool(name="sc", bufs=2, space="PSUM")
            scp = ph2.__enter__()
            ph2b = tc.tile_pool(name="acc", bufs=1, space="PSUM")
            accp = ph2b.__enter__()
            avt = accp.tile([33, Q], F32, name="avt")  # accumulator, 2 banks
            first = True
            for n in range(NCAM):
                for c in range(NKCH):
                    cw = 128 if c < KFULL else KTAIL
                    sc_ps = scp.tile([128, Q], F32, name="sc_ps", tag="sc_ps")
                    # lhsT = khT chunk [32, cw]
                    kap = khT[0:32, n, :]
                    for h in range(2):
                        nc.tensor.matmul(
                            sc_ps[0:cw, h * 512:(h + 1) * 512],
                            lhsT=kap[:, c * 128:c * 128 + cw],
                            rhs=qhT[0:32, n, h * 512:(h + 1) * 512],
                            start=True, stop=True)
                    et = eep.tile([128, Q], mybir.dt.bfloat16, name="et", tag="et")
                    nc.scalar.activation(out=et[0:cw, :], in_=sc_ps[0:cw, :],
                                         func=EXP,
                                         bias=lnrv[0:cw, n, c:c + 1],
                                         scale=rstdk[0:cw, n, c:c + 1])
                    for h in range(2):
                        nc.tensor.matmul(
                            avt[:, h * 512:(h + 1) * 512],
                            lhsT=vhE[0:cw, n, c, 0:33],
                            rhs=et[0:cw, h * 512:(h + 1) * 512],
                            start=first, stop=(n == NCAM - 1 and c == NKCH - 1))
                    first = False

            # ---- normalize + output projection ----
            avt_sb = finp.tile([33, Q], F32, name="avt_sb", tag="f1")
            nc.vector.tensor_copy(out=avt_sb, in_=avt)
            ph2b.__exit__(None, None, None)
            ph2.__exit__(None, None, None)
            ph3 = tc.tile_pool(name="stat2", bufs=1, space="PSUM")
            st2p = ph3.__enter__()
            den = row_split(avt_sb[32:33, :], 128, 8, "den", sml)
            rden = sml.tile([128, 8], F32R, name="rden")
            with nc.allow_low_precision(reason="denominator rounding to f32r is intentional"):
                nc.vector.reciprocal(out=rden, in_=den)
            rd_row = tm_join(rden, Q, "rd_row", F32R)
            rd_bc = st2p.tile([32, Q], F32, name="rd_bc")
            for h in range(2):
                nc.tensor.matmul(rd_bc[:, h * 512:(h + 1) * 512],
                                 lhsT=onesbc[:, 0:32],
                                 rhs=rd_row[:, h * 512:(h + 1) * 512],
                                 start=True, stop=True)
            anorm = finp.tile([32, Q], F32R, name="anorm", tag="f3")
            nc.vector.tensor_mul(out=anorm, in0=avt_sb[0:32, :], in1=rd_bc)
            nc.vector.tensor_scalar_add(out=anorm, in0=anorm,
                                        scalar1=wbv_t[0:32, :])

            zp_ps = st2p.tile([D, Q], F32, name="zp_ps")
            for h in range(2):
                nc.tensor.matmul(zp_ps[:, h * 512:(h + 1) * 512], lhsT=wp_t,
                                 rhs=anorm[:, h * 512:(h + 1) * 512],
                                 start=True, stop=True)
            zp_sb = finp.tile([D, Q], F32, name="zp_sb", tag="f1")
            nc.vector.tensor_copy(out=zp_sb, in_=zp_ps)
            zpart = dramp.tile([D, Q], F32, name="zpart")
            zred = dramp.tile([D, Q], F32, name="zred")
            nc.sync.dma_start(out=zpart, in_=zp_sb)
            ph3.__exit__(None, None, None)

            if collective:
                nc.gpsimd.collective_compute(
                    "AllReduce", mybir.AluOpType.add,
                    replica_groups=[[0, 1, 2, 3], [4, 5, 6, 7]],
                    ins=[zpart.opt()], outs=[zred.opt()],
                )
            else:
                nc.sync.dma_start(out=zred, in_=zpart)
            ph4 = tc.tile_pool(name="fps", bufs=1, space="PSUM")
            fpsp = ph4.__enter__()

            # ---- final: skip + pre-LN + MLP + post-LN (redundant x4) ----
            w1_t = consts.tile([D, 256], F32R, name="w1_t")
            nc.sync.dma_start(out=w1_t, in_=w1)
            w2_t = consts.tile([D, 2, D], F32R, name="w2_t")
            nc.sync.dma_start(out=w2_t, in_=w2)
            b1_t = consts.tile([D, 2], F32, name="b1_t")
            nc.sync.dma_start(out=b1_t, in_=b1.rearrange("h d one -> d (h one)"))
            b2_t = consts.tile([D, 1], F32, name="b2_t")
            nc.sync.dma_start(out=b2_t, in_=b2)
            bp_t = consts.tile([D, 1], F32, name="bp_t")
            nc.sync.dma_start(out=bp_t, in_=bp)
            preg_t = consts.tile([D, 1], F32, name="preg_t")
            nc.sync.dma_start(out=preg_t, in_=pre_g)
            preb_t = consts.tile([D, 1], F32, name="preb_t")
            nc.sync.dma_start(out=preb_t, in_=pre_b)
            postg_t = consts.tile([D, 1], F32, name="postg_t")
            nc.sync.dma_start(out=postg_t, in_=post_g)
            postb_t = consts.tile([D, 1], F32, name="postb_t")
            nc.sync.dma_start(out=postb_t, in_=post_b)
            skip_t = consts.tile([D, Q], F32, name="skip_t")
            nc.sync.dma_start(out=skip_t, in_=skipb)

            zt = finp.tile([D, Q], F32R, name="zt")
            nc.sync.dma_start(out=zt.bitcast(F32), in_=zred)
            nc.vector.tensor_add(out=zt, in0=zt, in1=skip_t)
            nc.vector.tensor_scalar_add(out=zt, in0=zt, scalar1=bp_t)

            def feat_ln(src, gain, bias_, dst_dt, dst_name):
                """LayerNorm across partitions (d) of src [128, Q]."""
                s2 = finp.tile([D, Q], F32R, name=dst_name + "_s2",
                               tag="f2")
                nc.vector.tensor_mul(out=s2, in0=src, in1=src)
                srow = rows.tile([1, Q], F32, name=dst_name + "_srow",
                                 tag="row")
                nc.gpsimd.tensor_reduce(out=srow, in_=src,
                                        axis=mybir.AxisListType.C,
                                        op=mybir.AluOpType.add)
                s2row = rows.tile([1, Q], F32, name=dst_name + "_s2row",
                                  tag="row")
                nc.gpsimd.tensor_reduce(out=s2row, in_=s2,
                                        axis=mybir.AxisListType.C,
                                        op=mybir.AluOpType.add)
                mu_tm = row_split(srow, 128, 8, "ln_mu", sml)
                ms_tm = row_split(s2row, 128, 8, "ln_ms", sml)
                # mu = sum/128 ; var = sumsq/128 - mu^2
                mu2 = sml.tile([128, 8], F32R, name=dst_name + "_mu2",
                               tag="ln_mu2")
                nc.vector.tensor_scalar_mul(out=mu2, in0=mu_tm,
                                            scalar1=1.0 / 128.0)
                v = sml.tile([128, 8], F32, name=dst_name + "_v", tag="ln_v")
                nc.vector.tensor_mul(out=v, in0=mu2, in1=mu2)
                v2 = sml.tile([128, 8], F32, name=dst_name + "_v2",
                              tag="ln_v2")
                nc.vector.tensor_scalar_mul(out=v2, in0=ms_tm,
                                            scalar1=1.0 / 128.0)
                nc.vector.tensor_sub(out=v, in0=v2, in1=v)
                nc.scalar.activation(out=v, in_=v, func=LN_, bias=eps_t,
                                     scale=1.0)
                vexp = sml.tile([128, 8], F32, name=dst_name + "_ve",
                                tag="ln_ve")
                nc.scalar.activation(out=vexp, in_=v, func=EXP, bias=0.0,
                                     scale=-0.5)
                rs_tm = sml.tile([128, 8], F32R, name=dst_name + "_rs",
                                 tag="ln_rs")
                nc.vector.tensor_copy(out=rs_tm, in_=vexp)
                # rows back
                mu_row = tm_join(mu2, Q, dst_name + "_mur", F32R)
                rs_row = tm_join(rs_tm, Q, dst_name + "_rsr", F32R)
                mu_bc = fpsp.tile([D, Q], F32, name=dst_name + "_mubc",
                                  tag="ln_mubc")
                rs_bc = fpsp.tile([D, Q], F32, name=dst_name + "_rsbc",
                                  tag="ln_rsbc")
                for h in range(2):
                    nc.tensor.matmul(mu_bc[:, h * 512:(h + 1) * 512],
                                     lhsT=onesbc,
                                     rhs=mu_row[:, h * 512:(h + 1) * 512],
                                     start=True, stop=True)
                    nc.tensor.matmul(rs_bc[:, h * 512:(h + 1) * 512],
                                     lhsT=onesbc,
                                     rhs=rs_row[:, h * 512:(h + 1) * 512],
                                     start=True, stop=True)
                zc = finp.tile([D, Q], F32, name=dst_name + "_zc",
                               tag="f2")
                nc.vector.tensor_sub(out=zc, in0=src, in1=mu_bc)
                dst = finp.tile([D, Q], dst_dt, name=dst_name, tag="lndst")
                nc.vector.tensor_mul(out=dst, in0=zc, in1=rs_bc)
                nc.vector.tensor_scalar_mul(out=dst, in0=dst, scalar1=gain)
                nc.vector.tensor_scalar_add(out=dst, in0=dst, scalar1=bias_)
                return dst

            zhat = feat_ln(zt, preg_t, preb_t, F32R, "zhat")  # tag lndst

            # MLP: h^T = gelu(W1^T zhat + b1)
            gel = finp.tile([D, 2, Q], F32R, name="gel")
            for f in range(2):
                h_ps = fpsp.tile([D, Q], F32, name="h_ps", tag="h_ps")
                for h in range(2):
                    nc.tensor.matmul(h_ps[:, h * 512:(h + 1) * 512],
                                     lhsT=w1_t[:, f * 128:(f + 1) * 128],
                                     rhs=zhat[:, h * 512:(h + 1) * 512],
                                     start=True, stop=True)
                nc.scalar.activation(out=gel[:, f, :], in_=h_ps, func=GELU,
                                     bias=b1_t[:, f:f + 1], scale=1.0)
            o2_ps = fpsp.tile([D, Q], F32, name="o2_ps")
            for f in range(2):
                for h in range(2):
                    nc.tensor.matmul(o2_ps[:, h * 512:(h + 1) * 512],
                                     lhsT=w2_t[:, f, :],
                                     rhs=gel[:, f, h * 512:(h + 1) * 512],
                                     start=(f == 0), stop=(f == 1))
            res = finp.tile([D, Q], F32R, name="res")
            nc.vector.tensor_scalar_add(out=res, in0=o2_ps, scalar1=b2_t)
            nc.vector.tensor_add(out=res, in0=res, in1=zhat)

            final = feat_ln(res, postg_t, postb_t, F32, "final")
            nc.sync.dma_start(out=out, in_=final)
            ph4.__exit__(None, None, None)

    if split:
        _split_sync_waits(nc)
    return nc


# ---------------------------------------------------------------------------
def _prep_core_inputs(b, m, q, k, v, skip, q_ln_g, q_ln_b, Wq, bq, k_ln_g,
                      k_ln_b, Wk, bk, v_ln_g, v_ln_b, Wv, bv, Wp, bp,
                      pre_g, pre_b, W1, b1, W2, b2, post_g, post_b):
    f32 = np.float32
    sl = slice(m * DH, (m + 1) * DH)

    def fold(Wm, g):
        wg = (g[:, None] * Wm)
        return (wg - wg.sum(0, keepdims=True) / 128.0).astype(f32)

    wq_ext = np.zeros((D, 33), f32)
    wq_ext[:, 0:32] = SCALE * fold(Wq[:, sl], q_ln_g)
    wq_ext[:, 32] = 1.0 / 128.0
    wk_ext = np.zeros((D, 33), f32)
    wk_ext[:, 0:32] = fold(Wk[:, sl], k_ln_g)
    wk_ext[:, 32] = 1.0 / 128.0
    wv_ext = np.zeros((D, 34), f32)
    wv_ext[:, 0:32] = fold(Wv[:, sl], v_ln_g)
    wv_ext[:, 33] = 1.0 / 128.0

    wbq = (SCALE * (Wq[:, sl].T @ q_ln_b)).astype(f32).reshape(32, 1)
    wbv = np.zeros((33, 1), f32)
    wbv[0:32, 0] = Wv[:, sl].T @ v_ln_b

    return {
        "xq": np.ascontiguousarray(q[b].reshape(NCAM, D, Q), f32),
        "xk": np.ascontiguousarray(k[b].reshape(NCAM, D, KC), f32),
        "xv": np.ascontiguousarray(v[b].reshape(NCAM, D, KC), f32),
        "wq_ext": wq_ext, "wk_ext": wk_ext, "wv_ext": wv_ext,
        "wbq": wbq, "wbv": wbv,
        "wp": np.ascontiguousarray(Wp[sl, :], f32),
        "bp": bp.astype(f32).reshape(D, 1),
        "skipb": np.ascontiguousarray(skip[b].reshape(D, Q), f32),
        "w1": W1.astype(f32),
        "b1": b1.astype(f32).reshape(2, D, 1),
        "w2": np.ascontiguousarray(
            W2.reshape(2, D, D).transpose(1, 0, 2), f32),
        "b2": b2.astype(f32).reshape(D, 1),
        "pre_g": pre_g.astype(f32).reshape(D, 1),
        "pre_b": pre_b.astype(f32).reshape(D, 1),
        "post_g": post_g.astype(f32).reshape(D, 1),
        "post_b": post_b.astype(f32).reshape(D, 1),
        "onesv": np.ones((1, D), f32),
    }


def kernel(**inputs):
    if "nc" not in _cached:
        _cached["nc"] = _build_program()
    nc = _cached["nc"]
    args = {kk: np.asarray(vv) for kk, vv in inputs.items()}
    in_maps = [_prep_core_inputs(c // 4, c % 4, **args) for c in range(N_CORES)]
    res = run_bass_kernel_spmd(nc, in_maps, core_ids=list(range(N_CORES)))
    out = np.stack([res.results[0]["out"], res.results[4]["out"]])
    return out.reshape(B, D, 32, 32)



# revision 7
# speedup vs baseline: 1.8094x; 1.8094x over previous
"""BEV cross-attention kernel for Trainium2, 8-core SPMD.

Shard: core c handles (batch b=c//4, query slice r=c%4 of 256 BEV queries),
computing ALL 4 heads for its queries. Keys/values (6 cams x 1680) are
replicated per core. No collectives: each core's output is a disjoint
[D, 256] token slice; the host concatenates.

Layout: feature-major ("S^T") attention - scores [keys=120p, (head, q)=1024f]
so softmax exp runs on ScalarE with per-partition (per-key) scale=rstd_k and
bias=ln(rstd_v) (K/V LayerNorms folded through the exp; shared by all heads).
LN means fold into centered projection weights host-side; the softmax
denominator rides the PV matmul as a per-head ones column of V. No max
subtraction (logits are small by construction).

Engine budget: ScalarE does the 84 exps (the wall, ~88us); PE does all
projections + QK/PV in bf16 (1 cyc/col); DVE does squares (bf16 2x) and
evacs; Pool does V evacs, cross-partition reduces and broadcasts. Per-token
LN stats are produced token-major directly by 1-col PE matmuls against a
ones vector (no DRAM bounces anywhere).
"""
import numpy as np
import ml_dtypes

import concourse.bass as bass
import concourse.bass_isa as bass_isa
import concourse.mybir as mybir
import concourse.tile as tile
from concourse.bass_utils import run_bass_kernel_spmd

F32 = mybir.dt.float32
F32R = mybir.dt.float32r
BF16 = mybir.dt.bfloat16

HEADS, DH, D = 4, 32, 128
B, NCAM = 2, 6
Q = 32 * 32            # 1024 BEV queries per batch
QS = Q // 4            # 256 queries per core
KC = 28 * 60           # 1680 keys per camera
CW = 120               # key chunk width: 1680 = 14 * 120, no tail
NKCH = KC // CW        # 14
N_CORES = 8
EPS = 1e-5
SCALE = DH ** -0.5

_cached = {}


# ---------------------------------------------------------------------------
# walrus compat: this container's walrus rejects instructions carrying more
# than one semaphore wait; move excess waits onto same-engine NoOps.
_COMPUTE_ENGINES = None
_nopctr = [0]


def _split_sync_waits(nc, limit=1):
    global _COMPUTE_ENGINES
    if _COMPUTE_ENGINES is None:
        _COMPUTE_ENGINES = {
            mybir.EngineType.PE, mybir.EngineType.Activation,
            mybir.EngineType.Pool, mybir.EngineType.DVE, mybir.EngineType.SP,
        }
    for f in nc.m.functions:
        for bb in f.blocks:
            out, changed = [], False
            for inst in bb.instructions:
                si = inst.sync_info
                if (si is not None and len(si.on_wait) > limit
                        and inst.engine in _COMPUTE_ENGINES):
                    waits = list(si.on_wait)
                    n_extra = len(waits) - limit
                    for i in range(0, n_extra, limit):
                        nop = mybir.InstNoOp(name=f"wait-split-{_nopctr[0]}")
                        _nopctr[0] += 1
                        nop.engine = inst.engine
                        nop.sync_info = mybir.SyncInfo(
                            on_wait=waits[i:min(i + limit, n_extra)], on_update=[])
                        out.append(nop)
                    si.on_wait = waits[n_extra:]
                    changed = True
                out.append(inst)
            if changed:
                bb.instructions = out


# ---------------------------------------------------------------------------
def _build_program(split=True, collective=True, n_dev=N_CORES):
    nc = bass.Bass("TRN2", target_bir_lowering=False, debug=False,
                   num_devices=n_dev)

    xq = nc.dram_tensor("xq", [NCAM, D, QS], BF16, kind="ExternalInput").ap()
    xk = nc.dram_tensor("xk", [NCAM, D, KC], BF16, kind="ExternalInput").ap()
    xv = nc.dram_tensor("xv", [NCAM, D, KC], BF16, kind="ExternalInput").ap()
    # packed bf16 weights: [wq 128 | wk 128 | wv_ext 132 | w1 256 | w2 256]
    wcat = nc.dram_tensor("wcat", [D, 900], BF16, kind="ExternalInput").ap()
    wp = nc.dram_tensor("wp", [DH, HEADS, D], F32R, kind="ExternalInput").ap()
    # packed f32 per-feature consts:
    # [wbq_pairA, wbq_pairB, bp', b1_0, b1_1, b2, pre_g, pre_b, post_g,
    #  post_b] (wbq pair columns hold heads 0-1 / 2-3 in partitions 0..63)
    fcon = nc.dram_tensor("fcon", [D, 10], F32, kind="ExternalInput").ap()
    skipb = nc.dram_tensor("skipb", [D, QS], F32, kind="ExternalInput").ap()

    out = nc.dram_tensor("out", [D, QS], F32, kind="ExternalOutput").ap()

    EXP = mybir.ActivationFunctionType.Exp
    LN_ = mybir.ActivationFunctionType.Ln
    SQRT = mybir.ActivationFunctionType.Sqrt
    GELU = mybir.ActivationFunctionType.Gelu
    ADD = mybir.AluOpType.add
    MULT = mybir.AluOpType.mult

    with tile.TileContext(nc) as tc:
        with tc.tile_pool(name="consts", bufs=1) as consts, \
             tc.tile_pool(name="loads", bufs=2) as loads, \
             tc.tile_pool(name="sq", bufs=2) as sqp, \
             tc.tile_pool(name="kv", bufs=2) as kvp, \
             tc.tile_pool(name="sml", bufs=2) as sml, \
             tc.tile_pool(name="ee", bufs=3) as eep, \
             tc.tile_pool(name="fin", bufs=1) as finp:

            # ---- constants (one DMA each for the packed tensors) ----
            wcat_t = consts.tile([D, 900], BF16, name="wcat_t")
            nc.sync.dma_start(out=wcat_t, in_=wcat)
            wq_t = wcat_t[:, 0:128]
            wk_t = wcat_t[:, 128:256]
            wv_t = wcat_t[:, 256:388]          # [D, 4*33]
            w1_t = wcat_t[:, 388:644]
            w2_t = wcat_t[:, 644:900].rearrange("p (f d) -> p f d", f=2)
            wp_t = consts.tile([DH, HEADS, D], F32R, name="wp_t")
            nc.sync.dma_start(out=wp_t, in_=wp)
            fcon_t = consts.tile([D, 10], F32, name="fcon_t")
            nc.sync.dma_start(out=fcon_t, in_=fcon)
            wbq_ab = fcon_t[:, 0:2]
            bpp_t = fcon_t[:, 2:3]
            b1_t = fcon_t[:, 3:5]
            b2_t = fcon_t[:, 5:6]
            preg_t = fcon_t[:, 6:7]
            preb_t = fcon_t[:, 7:8]
            postg_t = fcon_t[:, 8:9]
            postb_t = fcon_t[:, 9:10]
            skip_t = consts.tile([D, QS], F32, name="skip_t")
            nc.sync.dma_start(out=skip_t, in_=skipb)

            eps_t = consts.tile([D, 1], F32, name="eps_t")
            nc.vector.memset(eps_t, EPS)
            onesb = consts.tile([D, 2], BF16, name="onesb")  # [1 | 1/128]
            nc.vector.memset(onesb[:, 0:1], 1.0)
            nc.vector.memset(onesb[:, 1:2], 1.0 / 128.0)
            onesr = consts.tile([D, 2], F32R, name="onesr")  # [1 | 1/128]
            nc.vector.memset(onesr[:, 0:1], 1.0)
            nc.vector.memset(onesr[:, 1:2], 1.0 / 128.0)

            # ---- PSUM pools ----
            # banks: sc 2x2 + avt 2 + kp 1 + shared proj 1 = 8
            scpool = tc.tile_pool(name="scp", bufs=2, space="PSUM")
            scp = scpool.__enter__()
            accpool = tc.tile_pool(name="accp", bufs=1, space="PSUM")
            accp = accpool.__enter__()
            projpool = tc.tile_pool(name="projp", bufs=1, space="PSUM")
            projp = projpool.__enter__()

            avt = accp.tile([33, HEADS, QS], F32, name="avt")      # 2 banks
            kp_ps = projp.tile([64, 420], F32, name="kp_ps")       # 1 bank
            # shared 1-bank tile: vp [0:132] | st [132:188] | qp [256:512]
            sh_ps = projp.tile([D, 512], F32, name="sh_ps")        # 1 bank
            vp_ps = sh_ps[0:CW, 0:132]
            st_ps = sh_ps[0:CW, 132:188].rearrange("p (j c) -> p j c", j=4)
            qp_ps = sh_ps[0:64, 256:512]

            # ---- per-camera phase 1: load, square, project, stats ----
            def phase1(n):
                xk_t = loads.tile([D, KC], BF16, name="xk_t", tag="xk")
                nc.sync.dma_start(out=xk_t, in_=xk[n])
                xv_t = loads.tile([D, KC], BF16, name="xv_t", tag="xv")
                nc.sync.dma_start(out=xv_t, in_=xv[n])
                xq_t = loads.tile([D, QS], BF16, name="xq_t", tag="xq")
                nc.sync.dma_start(out=xq_t, in_=xq[n])

                x2k = sqp.tile([D, KC], BF16, name="x2k", tag="x2k")
                nc.vector.tensor_mul(out=x2k, in0=xk_t, in1=xk_t)
                x2v = sqp.tile([D, KC], BF16, name="x2v", tag="x2v")
                nc.vector.tensor_mul(out=x2v, in0=xv_t, in1=xv_t)
                x2q = sqp.tile([D, QS], BF16, name="x2q", tag="x2q")
                nc.vector.tensor_mul(out=x2q, in0=xq_t, in1=xq_t)

                # K projection, feature-major, split in head pairs so
                # attention lhsT slices sit at base partition 0/32
                khT = [kvp.tile([64, KC], BF16, name=f"khT{p}",
                                tag=f"khT{p}") for p in range(2)]
                for j in range(4):
                    for p in range(2):
                        nc.tensor.matmul(
                            kp_ps[:, 0:420], lhsT=wk_t[:, p * 64:(p + 1) * 64],
                            rhs=xk_t[:, j * 420:(j + 1) * 420],
                            start=True, stop=True)
                        nc.vector.tensor_copy(
                            out=khT[p][:, j * 420:(j + 1) * 420],
                            in_=kp_ps[:, 0:420])

                # V projection, token-major [120, 4, 33] per chunk
                # (col 32 of each head block is 0 from wv_ext; memset to 1
                # afterwards: softmax denominator ride-along)
                vhE = kvp.tile([CW, NKCH, HEADS, 33], BF16, name="vhE",
                               tag="vhE")
                for c in range(NKCH):
                    xvc = xv_t[:, c * CW:(c + 1) * CW]
                    nc.tensor.matmul(vp_ps, lhsT=xvc, rhs=wv_t,
                                     start=True, stop=True)
                    nc.gpsimd.tensor_copy(
                        out=vhE[:, c, :, :].rearrange("p h d -> p (h d)"),
                        in_=vp_ps)
                    nc.gpsimd.memset(vhE[:, c, :, 32], 1.0)

                # token-major stats via 1-col matmuls:
                # st rows: 0=k-mean, 1=v-mean, 2=k-sumsq, 3=v-sumsq
                for c in range(NKCH):
                    xkc = xk_t[:, c * CW:(c + 1) * CW]
                    xvc = xv_t[:, c * CW:(c + 1) * CW]
                    x2kc = x2k[:, c * CW:(c + 1) * CW]
                    x2vc = x2v[:, c * CW:(c + 1) * CW]
                    nc.tensor.matmul(st_ps[:, 0, c:c + 1], lhsT=xkc,
                                     rhs=onesb[:, 1:2], start=True, stop=True)
                    nc.tensor.matmul(st_ps[:, 1, c:c + 1], lhsT=xvc,
                                     rhs=onesb[:, 1:2], start=True, stop=True)
                    nc.tensor.matmul(st_ps[:, 2, c:c + 1], lhsT=x2kc,
                                     rhs=onesb[:, 0:1], start=True, stop=True)
                    nc.tensor.matmul(st_ps[:, 3, c:c + 1], lhsT=x2vc,
                                     rhs=onesb[:, 0:1], start=True, stop=True)

                st_sb = sml.tile([CW, 4, NKCH], F32, name="st_sb", tag="st")
                nc.vector.tensor_copy(out=st_sb, in_=st_ps)
                mu2 = sml.tile([CW, 2, NKCH], F32, name="mu2", tag="mu2")
                nc.vector.tensor_mul(out=mu2, in0=st_sb[:, 0:2, :],
                                     in1=st_sb[:, 0:2, :])
                var2 = sml.tile([CW, 2, NKCH], F32, name="var2", tag="var2")
                nc.vector.tensor_scalar_mul(out=var2, in0=st_sb[:, 2:4, :],
                                            scalar1=1.0 / 128.0)
                nc.vector.tensor_sub(out=var2, in0=var2, in1=mu2)
                # ln(var+eps) for both K and V in one ScalarE op
                lnb = sml.tile([CW, 2, NKCH], F32, name="lnb", tag="lnb")
                nc.scalar.activation(out=lnb, in_=var2, func=LN_,
                                     bias=eps_t[0:CW, :], scale=1.0)
                rstdk = kvp.tile([CW, NKCH], F32, name="rstdk", tag="rstdk")
                nc.scalar.activation(out=rstdk, in_=lnb[:, 0, :], func=EXP,
                                     bias=0.0, scale=-0.5)
                lnrv = kvp.tile([CW, NKCH], F32, name="lnrv", tag="lnrv")
                nc.vector.tensor_scalar_mul(out=lnrv, in0=lnb[:, 1, :],
                                            scalar1=-0.5)

                # Q projection (head pairs) + per-query rstd
                qhT = [kvp.tile([64, QS], BF16, name=f"qhT{p}",
                                tag=f"qhT{p}") for p in range(2)]
                musum = sml.tile([1, QS], F32, name="musum", tag="musum")
                nc.gpsimd.tensor_reduce(out=musum, in_=xq_t,
                                        axis=mybir.AxisListType.C, op=ADD)
                sssum = sml.tile([1, QS], F32, name="sssum", tag="sssum")
                nc.gpsimd.tensor_reduce(out=sssum, in_=x2q,
                                        axis=mybir.AxisListType.C, op=ADD)
                muq = sml.tile([1, QS], F32, name="muq", tag="muq")
                nc.vector.tensor_scalar_mul(out=muq, in0=musum,
                                            scalar1=1.0 / 128.0)
                mu2q = sml.tile([1, QS], F32, name="mu2q", tag="mu2q")
                nc.vector.tensor_mul(out=mu2q, in0=muq, in1=muq)
                varq = sml.tile([1, QS], F32, name="varq", tag="varq")
                nc.vector.tensor_scalar_mul(out=varq, in0=sssum,
                                            scalar1=1.0 / 128.0)
                nc.vector.tensor_sub(out=varq, in0=varq, in1=mu2q)
                sdq = sml.tile([1, QS], F32, name="sdq", tag="sdq")
                nc.scalar.activation(out=sdq, in_=varq, func=SQRT,
                                     bias=eps_t[0:1, :], scale=1.0)
                rqrow = sml.tile([1, QS], F32, name="rqrow", tag="rqrow")
                nc.vector.reciprocal(out=rqrow, in_=sdq)
                rqbc = sml.tile([64, QS], F32, name="rqbc", tag="rqbc")
                nc.gpsimd.partition_broadcast(rqbc, rqrow, channels=64)
                for p in range(2):
                    nc.tensor.matmul(qp_ps,
                                     lhsT=wq_t[:, p * 64:(p + 1) * 64],
                                     rhs=xq_t, start=True, stop=True)
                    nc.vector.tensor_mul(out=qhT[p], in0=qp_ps, in1=rqbc)
                    nc.vector.tensor_scalar_add(
                        out=qhT[p], in0=qhT[p],
                        scalar1=wbq_ab[0:64, p:p + 1])
                return khT, vhE, rstdk, lnrv, qhT

            # ---- attention for one camera ----
            def attention(n, cam):
                khT, vhE, rstdk, lnrv, qhT = cam
                for c in range(NKCH):
                    sc_ps = scp.tile([CW, HEADS, QS], F32, name="sc_ps",
                                     tag="sc")
                    for h in range(HEADS):
                        p, hh = divmod(h, 2)
                        nc.tensor.matmul(
                            sc_ps[:, h, :],
                            lhsT=khT[p][hh * DH:(hh + 1) * DH,
                                        c * CW:(c + 1) * CW],
                            rhs=qhT[p][hh * DH:(hh + 1) * DH, :],
                            start=True, stop=True)
                    et = eep.tile([CW, HEADS, QS], BF16, name="et", tag="et")
                    nc.scalar.activation(out=et, in_=sc_ps, func=EXP,
                                         bias=lnrv[:, c:c + 1],
                                         scale=rstdk[:, c:c + 1])
                    first = (n == 0 and c == 0)
                    last = (n == NCAM - 1 and c == NKCH - 1)
                    for h in range(HEADS):
                        nc.tensor.matmul(
                            avt[:, h, :],
                            lhsT=vhE[:, c, h, 0:33],
                            rhs=et[:, h, :],
                            start=first, stop=last)

            # ---- pipelined schedule: proj(n+1) issued before attn(n) ----
            cams = [phase1(0)]
            for n in range(NCAM):
                if n + 1 < NCAM:
                    cams.append(phase1(n + 1))
                attention(n, cams[n])

            # ---- tail: normalize, project, skip, LN, MLP, LN ----
            avt_sb = finp.tile([33, HEADS, QS], F32, name="avt_sb")
            nc.vector.tensor_copy(out=avt_sb, in_=avt)
            rden = finp.tile([1, HEADS, QS], F32, name="rden")
            nc.vector.reciprocal(out=rden, in_=avt_sb[32:33, :, :])
            rdbc = finp.tile([DH, HEADS, QS], F32, name="rdbc")
            nc.gpsimd.partition_broadcast(
                rdbc.rearrange("p h q -> p (h q)"),
                rden.rearrange("p h q -> p (h q)"), channels=DH)
            anorm = finp.tile([DH, HEADS, QS], F32R, name="anorm")
            nc.vector.tensor_mul(out=anorm, in0=avt_sb[0:32, :, :], in1=rdbc)

            projpool.__exit__(None, None, None)
            accpool.__exit__(None, None, None)
            scpool.__exit__(None, None, None)
            fpool = tc.tile_pool(name="fps", bufs=1, space="PSUM")
            fps = fpool.__enter__()

            zp_ps = fps.tile([D, QS], F32, name="zp_ps")
            for h in range(HEADS):
                nc.tensor.matmul(zp_ps, lhsT=wp_t[:, h, :],
                                 rhs=anorm[:, h, :],
                                 start=(h == 0), stop=(h == HEADS - 1))
            zt = finp.tile([D, QS], F32R, name="zt")
            nc.vector.tensor_add(out=zt, in0=zp_ps, in1=skip_t)
            nc.vector.tensor_scalar_add(out=zt, in0=zt, scalar1=bpp_t)

            row_ps = fps.tile([1, 2, QS], F32, name="row_ps")

            def feat_ln(src, gain, bias_, dst_dt, nm):
                """LayerNorm across partitions (d) of src [128, QS]."""
                s2 = finp.tile([D, QS], F32R, name=nm + "_s2", tag="ln_s2")
                nc.vector.tensor_mul(out=s2, in0=src, in1=src)
                nc.tensor.matmul(row_ps[:, 0, :], lhsT=onesr[:, 1:2],
                                 rhs=src, start=True, stop=True)
                nc.tensor.matmul(row_ps[:, 1, :], lhsT=onesr[:, 0:1],
                                 rhs=s2, start=True, stop=True)
                murow = sml.tile([1, QS], F32, name=nm + "_mu", tag="ln_mu")
                nc.vector.tensor_copy(out=murow, in_=row_ps[:, 0, :])
                m2 = sml.tile([1, QS], F32, name=nm + "_m2", tag="ln_m2")
                nc.vector.tensor_mul(out=m2, in0=murow, in1=murow)
                vr = sml.tile([1, QS], F32, name=nm + "_vr", tag="ln_vr")
                nc.vector.tensor_scalar_mul(out=vr, in0=row_ps[:, 1, :],
                                            scalar1=1.0 / 128.0)
                nc.vector.tensor_sub(out=vr, in0=vr, in1=m2)
                sd = sml.tile([1, QS], F32, name=nm + "_sd", tag="ln_sd")
                nc.scalar.activation(out=sd, in_=vr, func=SQRT,
                                     bias=eps_t[0:1, :], scale=1.0)
                rs = sml.tile([1, QS], F32, name=nm + "_rs", tag="ln_rs")
                nc.vector.reciprocal(out=rs, in_=sd)
                mubc = sml.tile([D, QS], F32, name=nm + "_mubc", tag="ln_mb")
                nc.gpsimd.partition_broadcast(mubc, murow, channels=D)
                rsbc = sml.tile([D, QS], F32, name=nm + "_rsbc", tag="ln_rb")
                nc.gpsimd.partition_broadcast(rsbc, rs, channels=D)
                zc = finp.tile([D, QS], F32R, name=nm + "_zc", tag="ln_zc")
                nc.vector.tensor_sub(out=zc, in0=src, in1=mubc)
                zm = finp.tile([D, QS], F32R, name=nm + "_zm", tag="ln_zm")
                nc.vector.tensor_mul(out=zm, in0=zc, in1=rsbc)
                dst = finp.tile([D, QS], dst_dt, name=nm, tag="ln_dst")
                nc.vector.tensor_scalar(out=dst, in0=zm, scalar1=gain,
                                        scalar2=bias_, op0=MULT, op1=ADD)
                return dst

            zhat = feat_ln(zt, preg_t, preb_t, BF16, "zhat")

            h_ps = fps.tile([D, 2, QS], F32, name="h_ps")
            for f in range(2):
                nc.tensor.matmul(h_ps[:, f, :],
                                 lhsT=w1_t[:, f * D:(f + 1) * D],
                                 rhs=zhat, start=True, stop=True)
            gel = finp.tile([D, 2, QS], BF16, name="gel")
            for f in range(2):
                nc.scalar.activation(out=gel[:, f, :], in_=h_ps[:, f, :],
                                     func=GELU, bias=b1_t[:, f:f + 1],
                                     scale=1.0)
            o2_ps = fps.tile([D, QS], F32, name="o2_ps")
            for f in range(2):
                nc.tensor.matmul(o2_ps, lhsT=w2_t[:, f, :], rhs=gel[:, f, :],
                                 start=(f == 0), stop=(f == 1))
            res = finp.tile([D, QS], F32R, name="res")
            nc.vector.tensor_scalar_add(out=res, in0=o2_ps, scalar1=b2_t)
            nc.vector.tensor_add(out=res, in0=res, in1=zhat)

            final = feat_ln(res, postg_t, postb_t, F32, "final")
            nc.sync.dma_start(out=out, in_=final)
            fpool.__exit__(None, None, None)

    if split:
        _split_sync_waits(nc)
    return nc


# ---------------------------------------------------------------------------
def _prep_core_inputs(b, r, q, k, v, skip, q_ln_g, q_ln_b, Wq, bq, k_ln_g,
                      k_ln_b, Wk, bk, v_ln_g, v_ln_b, Wv, bv, Wp, bp,
                      pre_g, pre_b, W1, b1, W2, b2, post_g, post_b):
    f32 = np.float32
    bf16 = ml_dtypes.bfloat16

    def fold(W, g):
        wg = g[:, None] * W
        return (wg - wg.sum(0, keepdims=True) / 128.0).astype(f32)

    wq_all = SCALE * fold(Wq, q_ln_g)                      # [D, 128]
    wk_all = fold(Wk, k_ln_g)                              # [D, 128]
    wv_f = fold(Wv, v_ln_g)                                # [D, 128]
    wv_ext = np.zeros((D, HEADS, 33), f32)
    wv_ext[:, :, 0:32] = wv_f.reshape(D, HEADS, DH)
    wcat = np.concatenate([
        wq_all, wk_all, wv_ext.reshape(D, HEADS * 33),
        W1.astype(f32),
        W2.reshape(2, D, D).transpose(1, 0, 2).reshape(D, 2 * D),
    ], axis=1).astype(bf16)

    # bias folding: q_ln_b -> wbq (added to q-heads); v_ln_b -> bp'
    # (rides through attention as a constant, then Wp); k_ln_b drops out
    # (adds a per-query constant to all logits -> softmax invariant).
    wbq = (SCALE * (Wq.T @ q_ln_b)).astype(f32)            # [128]
    wbv = Wv.T @ v_ln_b                                    # [128]
    bpp = bp + Wp.T @ wbv                                  # [D]
    pad = np.zeros(64, f32)
    fcon = np.stack([
        np.concatenate([wbq[0:64], pad]),
        np.concatenate([wbq[64:128], pad]),
        bpp, b1[0:D], b1[D:2 * D], b2,
        pre_g, pre_b, post_g, post_b,
    ], axis=1).astype(f32)

    sl = slice(r * QS, (r + 1) * QS)
    return {
        "xq": np.ascontiguousarray(
            q[b].reshape(NCAM, D, Q)[:, :, sl]).astype(bf16),
        "xk": np.ascontiguousarray(k[b].reshape(NCAM, D, KC)).astype(bf16),
        "xv": np.ascontiguousarray(v[b].reshape(NCAM, D, KC)).astype(bf16),
        "wcat": wcat,
        "wp": np.ascontiguousarray(
            Wp.reshape(HEADS, DH, D).transpose(1, 0, 2), f32),
        "fcon": fcon,
        "skipb": np.ascontiguousarray(skip[b].reshape(D, Q)[:, sl], f32),
    }


def kernel(**inputs):
    if "nc" not in _cached:
        _cached["nc"] = _build_program()
    nc = _cached["nc"]
    args = {kk: np.asarray(vv) for kk, vv in inputs.items()}
    in_maps = [_prep_core_inputs(c // 4, c % 4, **args) for c in range(N_CORES)]
    res = run_bass_kernel_spmd(nc, in_maps, core_ids=list(range(N_CORES)))
    full = np.zeros((B, D, Q), np.float32)
    for c in range(N_CORES):
        b, r = c // 4, c % 4
        full[b][:, r * QS:(r + 1) * QS] = res.results[c]["out"]
    return full.reshape(B, D, 32, 32)


# revision 9
# speedup vs baseline: 1.8407x; 1.0173x over previous
"""BEV cross-attention kernel for Trainium2, 8-core SPMD.

Shard: core c handles (batch b=c//4, query slice r=c%4 of 256 BEV queries),
computing ALL 4 heads for its queries. Keys/values (6 cams x 1680) are
replicated per core. No collectives: each core's output is a disjoint
[D, 256] token slice; the host concatenates.

Layout: feature-major ("S^T") attention - scores [keys=120p, (head, q)=1024f]
so softmax exp runs on ScalarE with per-partition (per-key) scale=rstd_k and
bias=ln(rstd_v) (K/V LayerNorms folded through the exp; shared by all heads).
LN means fold into centered projection weights host-side; the softmax
denominator rides the PV matmul as a per-head ones column of V. No max
subtraction (logits are small by construction).

Engine budget: ScalarE does the 84 exps (the wall, ~88us); PE does all
projections + QK/PV in bf16 (1 cyc/col); DVE does squares (bf16 2x) and
evacs; Pool does V evacs, cross-partition reduces and broadcasts. Per-token
LN stats are produced token-major directly by 1-col PE matmuls against a
ones vector (no DRAM bounces anywhere).
"""
import numpy as np
import ml_dtypes

import concourse.bass as bass
import concourse.bass_isa as bass_isa
import concourse.mybir as mybir
import concourse.tile as tile
from concourse.bass_utils import run_bass_kernel_spmd

F32 = mybir.dt.float32
F32R = mybir.dt.float32r
BF16 = mybir.dt.bfloat16

HEADS, DH, D = 4, 32, 128
B, NCAM = 2, 6
Q = 32 * 32            # 1024 BEV queries per batch
QS = Q // 4            # 256 queries per core
KC = 28 * 60           # 1680 keys per camera
CW = 120               # key chunk width: 1680 = 14 * 120, no tail
NKCH = KC // CW        # 14
N_CORES = 8
EPS = 1e-5
SCALE = DH ** -0.5

_cached = {}


# ---------------------------------------------------------------------------
# walrus compat: this container's walrus rejects instructions carrying more
# than one semaphore wait; move excess waits onto same-engine NoOps.
_COMPUTE_ENGINES = None
_nopctr = [0]


def _split_sync_waits(nc, limit=1):
    global _COMPUTE_ENGINES
    if _COMPUTE_ENGINES is None:
        _COMPUTE_ENGINES = {
            mybir.EngineType.PE, mybir.EngineType.Activation,
            mybir.EngineType.Pool, mybir.EngineType.DVE, mybir.EngineType.SP,
        }
    for f in nc.m.functions:
        for bb in f.blocks:
            out, changed = [], False
            for inst in bb.instructions:
                si = inst.sync_info
                if (si is not None and len(si.on_wait) > limit
                        and inst.engine in _COMPUTE_ENGINES):
                    waits = list(si.on_wait)
                    n_extra = len(waits) - limit
                    for i in range(0, n_extra, limit):
                        nop = mybir.InstNoOp(name=f"wait-split-{_nopctr[0]}")
                        _nopctr[0] += 1
                        nop.engine = inst.engine
                        nop.sync_info = mybir.SyncInfo(
                            on_wait=waits[i:min(i + limit, n_extra)], on_update=[])
                        out.append(nop)
                    si.on_wait = waits[n_extra:]
                    changed = True
                out.append(inst)
            if changed:
                bb.instructions = out


# ---------------------------------------------------------------------------
def _build_program(split=True, collective=True, n_dev=N_CORES):
    nc = bass.Bass("TRN2", target_bir_lowering=False, debug=False,
                   num_devices=n_dev)

    xq = nc.dram_tensor("xq", [NCAM, D, QS], BF16, kind="ExternalInput").ap()
    xk = nc.dram_tensor("xk", [NCAM, D, KC], BF16, kind="ExternalInput").ap()
    xv = nc.dram_tensor("xv", [NCAM, D, KC], BF16, kind="ExternalInput").ap()
    # packed bf16 weights: [wq 128 | wk 128 | wv_ext 132 | w1 256 | w2 256]
    wcat = nc.dram_tensor("wcat", [D, 900], BF16, kind="ExternalInput").ap()
    wp = nc.dram_tensor("wp", [DH, HEADS, D], F32R, kind="ExternalInput").ap()
    # packed f32 per-feature consts:
    # [wbq_pairA, wbq_pairB, bp', b1_0, b1_1, b2, pre_g, pre_b, post_g,
    #  post_b] (wbq pair columns hold heads 0-1 / 2-3 in partitions 0..63)
    fcon = nc.dram_tensor("fcon", [D, 10], F32, kind="ExternalInput").ap()
    skipb = nc.dram_tensor("skipb", [D, QS], F32, kind="ExternalInput").ap()

    out = nc.dram_tensor("out", [D, QS], F32, kind="ExternalOutput").ap()

    EXP = mybir.ActivationFunctionType.Exp
    LN_ = mybir.ActivationFunctionType.Ln
    SQRT = mybir.ActivationFunctionType.Sqrt
    GELU = mybir.ActivationFunctionType.Gelu
    ADD = mybir.AluOpType.add
    MULT = mybir.AluOpType.mult

    with tile.TileContext(nc) as tc:
        with tc.tile_pool(name="consts", bufs=1) as consts, \
             tc.tile_pool(name="loads", bufs=2) as loads, \
             tc.tile_pool(name="sq", bufs=2) as sqp, \
             tc.tile_pool(name="kv", bufs=2) as kvp, \
             tc.tile_pool(name="sml", bufs=2) as sml, \
             tc.tile_pool(name="ee", bufs=3) as eep, \
             tc.tile_pool(name="fin", bufs=1) as finp:

            # ---- constants (one DMA each for the packed tensors) ----
            wcat_t = consts.tile([D, 900], BF16, name="wcat_t")
            nc.sync.dma_start(out=wcat_t, in_=wcat)
            wq_t = wcat_t[:, 0:128]
            wk_t = wcat_t[:, 128:256]
            wv_t = wcat_t[:, 256:388]          # [D, 4*33]
            w1_t = wcat_t[:, 388:644]
            w2_t = wcat_t[:, 644:900].rearrange("p (f d) -> p f d", f=2)
            wp_t = consts.tile([DH, HEADS, D], F32R, name="wp_t")
            nc.sync.dma_start(out=wp_t, in_=wp)
            fcon_t = consts.tile([D, 10], F32, name="fcon_t")
            nc.sync.dma_start(out=fcon_t, in_=fcon)
            wbq_ab = fcon_t[:, 0:2]
            bpp_t = fcon_t[:, 2:3]
            b1_t = fcon_t[:, 3:5]
            b2_t = fcon_t[:, 5:6]
            preg_t = fcon_t[:, 6:7]
            preb_t = fcon_t[:, 7:8]
            postg_t = fcon_t[:, 8:9]
            postb_t = fcon_t[:, 9:10]
            skip_t = consts.tile([D, QS], F32, name="skip_t")
            nc.sync.dma_start(out=skip_t, in_=skipb)

            eps_t = consts.tile([D, 1], F32, name="eps_t")
            nc.vector.memset(eps_t, EPS)
            onesb = consts.tile([D, 2], BF16, name="onesb")  # [1 | 1/128]
            nc.vector.memset(onesb[:, 0:1], 1.0)
            nc.vector.memset(onesb[:, 1:2], 1.0 / 128.0)
            onesr = consts.tile([D, 2], F32R, name="onesr")  # [1 | 1/128]
            nc.vector.memset(onesr[:, 0:1], 1.0)
            nc.vector.memset(onesr[:, 1:2], 1.0 / 128.0)
            ones_row = consts.tile([1, D], F32R, name="ones_row")
            nc.vector.memset(ones_row, 1.0)

            # ---- PSUM pools ----
            # banks: avt 2 + sc 2x2 + kp 1 + shared proj 1 = 8
            accpool = tc.tile_pool(name="accp", bufs=1, space="PSUM")
            accp = accpool.__enter__()
            scpool = tc.tile_pool(name="scp", bufs=2, space="PSUM")
            scp = scpool.__enter__()
            projpool = tc.tile_pool(name="projp", bufs=1, space="PSUM")
            projp = projpool.__enter__()

            avt = accp.tile([33, HEADS, QS], F32, name="avt")      # 2 banks
            kp_ps = projp.tile([64, 420], F32, name="kp_ps")       # 1 bank
            # shared 1-bank tile: vp [0:132] | st [132:188] | qp [256:512]
            sh_ps = projp.tile([D, 512], F32, name="sh_ps")        # 1 bank
            vp_ps = sh_ps[0:CW, 0:132]
            st_ps = sh_ps[0:CW, 132:188].rearrange("p (j c) -> p j c", j=4)
            qp_ps = sh_ps[0:64, 256:512]

            # ---- per-camera phase 1, split so the ScalarE ops (finish)
            # can be emitted mid-attention of the previous camera and never
            # block the exp stream ----
            def produce(n):
                xk_t = loads.tile([D, KC], BF16, name="xk_t", tag="xk")
                nc.sync.dma_start(out=xk_t, in_=xk[n])
                xv_t = loads.tile([D, KC], BF16, name="xv_t", tag="xv")
                nc.sync.dma_start(out=xv_t, in_=xv[n])
                xq_t = loads.tile([D, QS], BF16, name="xq_t", tag="xq")
                nc.sync.dma_start(out=xq_t, in_=xq[n])

                x2k = sqp.tile([D, KC], BF16, name="x2k", tag="x2k")
                nc.vector.tensor_mul(out=x2k, in0=xk_t, in1=xk_t)
                x2v = sqp.tile([D, KC], BF16, name="x2v", tag="x2v")
                nc.vector.tensor_mul(out=x2v, in0=xv_t, in1=xv_t)
                x2q = sqp.tile([D, QS], BF16, name="x2q", tag="x2q")
                nc.vector.tensor_mul(out=x2q, in0=xq_t, in1=xq_t)

                # token-major stats via 1-col matmuls:
                # st rows: 0=k-mean, 1=v-mean, 2=k-sumsq, 3=v-sumsq
                for c in range(NKCH):
                    xkc = xk_t[:, c * CW:(c + 1) * CW]
                    xvc = xv_t[:, c * CW:(c + 1) * CW]
                    x2kc = x2k[:, c * CW:(c + 1) * CW]
                    x2vc = x2v[:, c * CW:(c + 1) * CW]
                    nc.tensor.matmul(st_ps[:, 0, c:c + 1], lhsT=xkc,
                                     rhs=onesb[:, 1:2], start=True, stop=True)
                    nc.tensor.matmul(st_ps[:, 1, c:c + 1], lhsT=xvc,
                                     rhs=onesb[:, 1:2], start=True, stop=True)
                    nc.tensor.matmul(st_ps[:, 2, c:c + 1], lhsT=x2kc,
                                     rhs=onesb[:, 0:1], start=True, stop=True)
                    nc.tensor.matmul(st_ps[:, 3, c:c + 1], lhsT=x2vc,
                                     rhs=onesb[:, 0:1], start=True, stop=True)

                st_sb = sml.tile([CW, 4, NKCH], F32, name="st_sb", tag="st")
                nc.vector.tensor_copy(out=st_sb, in_=st_ps)
                mu2 = sml.tile([CW, 2, NKCH], F32, name="mu2", tag="mu2")
                nc.vector.tensor_mul(out=mu2, in0=st_sb[:, 0:2, :],
                                     in1=st_sb[:, 0:2, :])
                var2 = sml.tile([CW, 2, NKCH], F32, name="var2", tag="var2")
                nc.vector.tensor_scalar_mul(out=var2, in0=st_sb[:, 2:4, :],
                                            scalar1=1.0 / 128.0)
                nc.vector.tensor_sub(out=var2, in0=var2, in1=mu2)

                # q stats rows (Pool C-reduce, off the PE/Act path)
                musum = sml.tile([1, QS], F32, name="musum", tag="musum")
                nc.gpsimd.tensor_reduce(out=musum, in_=xq_t,
                                        axis=mybir.AxisListType.C, op=ADD)
                sssum = sml.tile([1, QS], F32, name="sssum", tag="sssum")
                nc.gpsimd.tensor_reduce(out=sssum, in_=x2q,
                                        axis=mybir.AxisListType.C, op=ADD)
                muq = sml.tile([1, QS], F32, name="muq", tag="muq")
                nc.vector.tensor_scalar_mul(out=muq, in0=musum,
                                            scalar1=1.0 / 128.0)
                mu2q = sml.tile([1, QS], F32, name="mu2q", tag="mu2q")
                nc.vector.tensor_mul(out=mu2q, in0=muq, in1=muq)
                varq = sml.tile([1, QS], F32, name="varq", tag="varq")
                nc.vector.tensor_scalar_mul(out=varq, in0=sssum,
                                            scalar1=1.0 / 128.0)
                nc.vector.tensor_sub(out=varq, in0=varq, in1=mu2q)

                # K projection, feature-major, split in head pairs so
                # attention lhsT slices sit at base partition 0/32
                khT = [kvp.tile([64, KC], BF16, name=f"khT{p}",
                                tag=f"khT{p}") for p in range(2)]
                for j in range(4):
                    for p in range(2):
                        nc.tensor.matmul(
                            kp_ps[:, 0:420], lhsT=wk_t[:, p * 64:(p + 1) * 64],
                            rhs=xk_t[:, j * 420:(j + 1) * 420],
                            start=True, stop=True)
                        nc.vector.tensor_copy(
                            out=khT[p][:, j * 420:(j + 1) * 420],
                            in_=kp_ps[:, 0:420])

                # V projection, token-major [120, 4, 33] per chunk
                # (col 32 of each head block is 0 from wv_ext; memset to 1
                # afterwards: softmax denominator ride-along)
                vhE = kvp.tile([CW, NKCH, HEADS, 33], BF16, name="vhE",
                               tag="vhE")
                for c in range(NKCH):
                    xvc = xv_t[:, c * CW:(c + 1) * CW]
                    nc.tensor.matmul(vp_ps, lhsT=xvc, rhs=wv_t,
                                     start=True, stop=True)
                    nc.gpsimd.tensor_copy(
                        out=vhE[:, c, :, :].rearrange("p h d -> p (h d)"),
                        in_=vp_ps)
                    nc.gpsimd.memset(vhE[:, c, :, 32], 1.0)
                return xq_t, var2, varq, khT, vhE

            def finish(n, prod):
                xq_t, var2, varq, khT, vhE = prod
                # ln(var+eps) for both K and V in one ScalarE op
                lnb = sml.tile([CW, 2, NKCH], F32, name="lnb", tag="lnb")
                nc.scalar.activation(out=lnb, in_=var2, func=LN_,
                                     bias=eps_t[0:CW, :], scale=1.0)
                rstdk = kvp.tile([CW, NKCH], F32, name="rstdk", tag="rstdk")
                nc.scalar.activation(out=rstdk, in_=lnb[:, 0, :], func=EXP,
                                     bias=0.0, scale=-0.5)
                lnrv = kvp.tile([CW, NKCH], F32, name="lnrv", tag="lnrv")
                nc.vector.tensor_scalar_mul(out=lnrv, in0=lnb[:, 1, :],
                                            scalar1=-0.5)

                sdq = sml.tile([1, QS], F32, name="sdq", tag="sdq")
                nc.scalar.activation(out=sdq, in_=varq, func=SQRT,
                                     bias=eps_t[0:1, :], scale=1.0)
                rqrow = sml.tile([1, QS], F32, name="rqrow", tag="rqrow")
                nc.vector.reciprocal(out=rqrow, in_=sdq)
                rqbc = sml.tile([64, QS], F32, name="rqbc", tag="rqbc")
                nc.gpsimd.partition_broadcast(rqbc, rqrow, channels=64)
                qhT = [kvp.tile([64, QS], BF16, name=f"qhT{p}",
                                tag=f"qhT{p}") for p in range(2)]
                for p in range(2):
                    nc.tensor.matmul(qp_ps,
                                     lhsT=wq_t[:, p * 64:(p + 1) * 64],
                                     rhs=xq_t, start=True, stop=True)
                    nc.vector.tensor_mul(out=qhT[p], in0=qp_ps, in1=rqbc)
                    nc.vector.tensor_scalar_add(
                        out=qhT[p], in0=qhT[p],
                        scalar1=wbq_ab[0:64, p:p + 1])
                return khT, vhE, rstdk, lnrv, qhT

            # ---- attention for one camera (chunk range) ----
            def attention(n, cam, c0, c1):
                khT, vhE, rstdk, lnrv, qhT = cam
                for c in range(c0, c1):
                    sc_ps = scp.tile([CW, HEADS, QS], F32, name="sc_ps",
                                     tag="sc")
                    for h in range(HEADS):
                        p, hh = divmod(h, 2)
                        nc.tensor.matmul(
                            sc_ps[:, h, :],
                            lhsT=khT[p][hh * DH:(hh + 1) * DH,
                                        c * CW:(c + 1) * CW],
                            rhs=qhT[p][hh * DH:(hh + 1) * DH, :],
                            start=True, stop=True)
                    et = eep.tile([CW, HEADS, QS], BF16, name="et", tag="et")
                    nc.scalar.activation(out=et, in_=sc_ps, func=EXP,
                                         bias=lnrv[:, c:c + 1],
                                         scale=rstdk[:, c:c + 1])
                    first = (n == 0 and c == 0)
                    last = (n == NCAM - 1 and c == NKCH - 1)
                    for h in range(HEADS):
                        nc.tensor.matmul(
                            avt[:, h, :],
                            lhsT=vhE[:, c, h, 0:33],
                            rhs=et[:, h, :],
                            start=first, stop=last)

            # ---- pipelined schedule: produce(n+1) and finish(n+1) are
            # emitted around the first half of attention(n) so no engine's
            # in-order queue ever blocks the exp stream ----
            cams = [None] * NCAM
            prod0 = produce(0)
            cams[0] = finish(0, prod0)
            prods = [None] * NCAM
            if NCAM > 1:
                prods[1] = produce(1)
            for n in range(NCAM):
                attention(n, cams[n], 0, 7)
                if n + 1 < NCAM:
                    cams[n + 1] = finish(n + 1, prods[n + 1])
                attention(n, cams[n], 7, NKCH)
                if n + 2 < NCAM:
                    prods[n + 2] = produce(n + 2)

            # ---- tail: normalize, project, skip, LN, MLP, LN ----
            # per-(head,query) denominator: reciprocal straight from PSUM,
            # broadcast via PE matmul (ones row), normalize from PSUM
            rden = finp.tile([1, HEADS, QS], F32R, name="rden")
            with nc.allow_low_precision(reason="f32r denominator"):
                nc.vector.reciprocal(out=rden, in_=avt[32:33, :, :])

            projpool.__exit__(None, None, None)
            scpool.__exit__(None, None, None)
            fpool = tc.tile_pool(name="fps", bufs=1, space="PSUM")
            fps = fpool.__enter__()
            rd_ps = fps.tile([DH, 512], F32, name="rd_ps")
            anorm = finp.tile([DH, HEADS, QS], F32R, name="anorm")
            rden_f = rden.rearrange("p h q -> p (h q)")
            anorm_f = anorm.rearrange("p h q -> p (h q)")
            avt_f = avt[0:32, :, :].rearrange("p h q -> p (h q)")
            for j in range(2):
                nc.tensor.matmul(rd_ps, lhsT=ones_row[:, 0:DH],
                                 rhs=rden_f[:, j * 512:(j + 1) * 512],
                                 start=True, stop=True)
                nc.vector.tensor_mul(out=anorm_f[:, j * 512:(j + 1) * 512],
                                     in0=avt_f[:, j * 512:(j + 1) * 512],
                                     in1=rd_ps)

            zo_ps = fps.tile([D, QS], F32, name="zo_ps", tag="zo")
            for h in range(HEADS):
                nc.tensor.matmul(zo_ps, lhsT=wp_t[:, h, :],
                                 rhs=anorm[:, h, :],
                                 start=(h == 0), stop=(h == HEADS - 1))
            zt = finp.tile([D, QS], F32R, name="zt")
            nc.vector.tensor_add(out=zt, in0=zo_ps, in1=skip_t)
            nc.vector.tensor_scalar_add(out=zt, in0=zt, scalar1=bpp_t)

            row_ps = fps.tile([1, 2, QS], F32, name="row_ps")
            mr_ps = fps.tile([D, 2, QS], F32, name="mr_ps")

            def feat_ln(src, gain, bias_, dst_dt, nm):
                """LayerNorm across partitions (d) of src [128, QS]."""
                s2 = finp.tile([D, QS], F32R, name=nm + "_s2", tag="ln_s2")
                nc.vector.tensor_mul(out=s2, in0=src, in1=src)
                nc.tensor.matmul(row_ps[:, 0, :], lhsT=onesr[:, 1:2],
                                 rhs=src, start=True, stop=True)
                nc.tensor.matmul(row_ps[:, 1, :], lhsT=onesr[:, 0:1],
                                 rhs=s2, start=True, stop=True)
                murow = sml.tile([1, QS], F32R, name=nm + "_mu", tag="ln_mu")
                nc.vector.tensor_copy(out=murow, in_=row_ps[:, 0, :])
                m2 = sml.tile([1, QS], F32, name=nm + "_m2", tag="ln_m2")
                nc.vector.tensor_mul(out=m2, in0=murow, in1=murow)
                vr = sml.tile([1, QS], F32, name=nm + "_vr", tag="ln_vr")
                nc.vector.tensor_scalar_mul(out=vr, in0=row_ps[:, 1, :],
                                            scalar1=1.0 / 128.0)
                nc.vector.tensor_sub(out=vr, in0=vr, in1=m2)
                sd = sml.tile([1, QS], F32, name=nm + "_sd", tag="ln_sd")
                nc.scalar.activation(out=sd, in_=vr, func=SQRT,
                                     bias=eps_t[0:1, :], scale=1.0)
                rs = sml.tile([1, QS], F32R, name=nm + "_rs", tag="ln_rs")
                with nc.allow_low_precision(reason="f32r rstd"):
                    nc.vector.reciprocal(out=rs, in_=sd)
                nc.tensor.matmul(mr_ps[:, 0, :], lhsT=ones_row,
                                 rhs=murow, start=True, stop=True)
                nc.tensor.matmul(mr_ps[:, 1, :], lhsT=ones_row,
                                 rhs=rs, start=True, stop=True)
                zc = finp.tile([D, QS], F32R, name=nm + "_zc", tag="ln_zc")
                nc.vector.tensor_sub(out=zc, in0=src, in1=mr_ps[:, 0, :])
                zm = finp.tile([D, QS], F32R, name=nm + "_zm", tag="ln_zm")
                nc.vector.tensor_mul(out=zm, in0=zc, in1=mr_ps[:, 1, :])
                dst = finp.tile([D, QS], dst_dt, name=nm, tag="ln_dst")
                nc.vector.tensor_scalar(out=dst, in0=zm, scalar1=gain,
                                        scalar2=bias_, op0=MULT, op1=ADD)
                return dst

            zhat = feat_ln(zt, preg_t, preb_t, BF16, "zhat")

            h_ps = fps.tile([D, 2, QS], F32, name="h_ps")
            for f in range(2):
                nc.tensor.matmul(h_ps[:, f, :],
                                 lhsT=w1_t[:, f * D:(f + 1) * D],
                                 rhs=zhat, start=True, stop=True)
            gel = finp.tile([D, 2, QS], BF16, name="gel")
            for f in range(2):
                nc.scalar.activation(out=gel[:, f, :], in_=h_ps[:, f, :],
                                     func=GELU, bias=b1_t[:, f:f + 1],
                                     scale=1.0)
            o2_ps = fps.tile([D, QS], F32, name="o2_ps", tag="zo")
            for f in range(2):
                nc.tensor.matmul(o2_ps, lhsT=w2_t[:, f, :], rhs=gel[:, f, :],
                                 start=(f == 0), stop=(f == 1))
            res = finp.tile([D, QS], F32R, name="res")
            nc.vector.tensor_scalar_add(out=res, in0=o2_ps, scalar1=b2_t)
            nc.vector.tensor_add(out=res, in0=res, in1=zhat)

            final = feat_ln(res, postg_t, postb_t, F32, "final")
            nc.sync.dma_start(out=out, in_=final)
            fpool.__exit__(None, None, None)
            accpool.__exit__(None, None, None)

    if split:
        _split_sync_waits(nc)
    return nc


# ---------------------------------------------------------------------------
def _prep_core_inputs(b, r, q, k, v, skip, q_ln_g, q_ln_b, Wq, bq, k_ln_g,
                      k_ln_b, Wk, bk, v_ln_g, v_ln_b, Wv, bv, Wp, bp,
                      pre_g, pre_b, W1, b1, W2, b2, post_g, post_b):
    f32 = np.float32
    bf16 = ml_dtypes.bfloat16

    def fold(W, g):
        wg = g[:, None] * W
        return (wg - wg.sum(0, keepdims=True) / 128.0).astype(f32)

    wq_all = SCALE * fold(Wq, q_ln_g)                      # [D, 128]
    wk_all = fold(Wk, k_ln_g)                              # [D, 128]
    wv_f = fold(Wv, v_ln_g)                                # [D, 128]
    wv_ext = np.zeros((D, HEADS, 33), f32)
    wv_ext[:, :, 0:32] = wv_f.reshape(D, HEADS, DH)
    wcat = np.concatenate([
        wq_all, wk_all, wv_ext.reshape(D, HEADS * 33),
        W1.astype(f32),
        W2.reshape(2, D, D).transpose(1, 0, 2).reshape(D, 2 * D),
    ], axis=1).astype(bf16)

    # bias folding: q_ln_b -> wbq (added to q-heads); v_ln_b -> bp'
    # (rides through attention as a constant, then Wp); k_ln_b drops out
    # (adds a per-query constant to all logits -> softmax invariant).
    wbq = (SCALE * (Wq.T @ q_ln_b)).astype(f32)            # [128]
    wbv = Wv.T @ v_ln_b                                    # [128]
    bpp = bp + Wp.T @ wbv                                  # [D]
    pad = np.zeros(64, f32)
    fcon = np.stack([
        np.concatenate([wbq[0:64], pad]),
        np.concatenate([wbq[64:128], pad]),
        bpp, b1[0:D], b1[D:2 * D], b2,
        pre_g, pre_b, post_g, post_b,
    ], axis=1).astype(f32)

    sl = slice(r * QS, (r + 1) * QS)
    return {
        "xq": np.ascontiguousarray(
            q[b].reshape(NCAM, D, Q)[:, :, sl]).astype(bf16),
        "xk": np.ascontiguousarray(k[b].reshape(NCAM, D, KC)).astype(bf16),
        "xv": np.ascontiguousarray(v[b].reshape(NCAM, D, KC)).astype(bf16),
        "wcat": wcat,
        "wp": np.ascontiguousarray(
            Wp.reshape(HEADS, DH, D).transpose(1, 0, 2), f32),
        "fcon": fcon,
        "skipb": np.ascontiguousarray(skip[b].reshape(D, Q)[:, sl], f32),
    }


def kernel(**inputs):
    if "nc" not in _cached:
        _cached["nc"] = _build_program()
    nc = _cached["nc"]
    args = {kk: np.asarray(vv) for kk, vv in inputs.items()}
    in_maps = [_prep_core_inputs(c // 4, c % 4, **args) for c in range(N_CORES)]
    res = run_bass_kernel_spmd(nc, in_maps, core_ids=list(range(N_CORES)))
    full = np.zeros((B, D, Q), np.float32)
    for c in range(N_CORES):
        b, r = c // 4, c % 4
        full[b][:, r * QS:(r + 1) * QS] = res.results[c]["out"]
    return full.reshape(B, D, 32, 32)


# revision 10
# speedup vs baseline: 1.8815x; 1.0222x over previous
"""BEV cross-attention kernel for Trainium2, 8-core SPMD.

Shard: core c handles (batch b=c//4, query slice r=c%4 of 256 BEV queries),
computing ALL 4 heads for its queries. Keys/values (6 cams x 1680) are
replicated per core. No collectives: each core's output is a disjoint
[D, 256] token slice; the host concatenates.

Layout: feature-major ("S^T") attention - scores [keys=120p, (head, q)=1024f]
so softmax exp runs on ScalarE with per-partition (per-key) scale=rstd_k and
bias=ln(rstd_v) (K/V LayerNorms folded through the exp; shared by all heads).
LN means fold into centered projection weights host-side; the softmax
denominator rides the PV matmul as a per-head ones column of V. No max
subtraction (logits are small by construction).

Engine budget: ScalarE does the 84 exps (the wall, ~88us); PE does all
projections + QK/PV in bf16 (1 cyc/col); DVE does squares (bf16 2x) and
evacs; Pool does V evacs, cross-partition reduces and broadcasts. Per-token
LN stats are produced token-major directly by 1-col PE matmuls against a
ones vector (no DRAM bounces anywhere).
"""
import numpy as np
import ml_dtypes

import concourse.bass as bass
import concourse.bass_isa as bass_isa
import concourse.mybir as mybir
import concourse.tile as tile
from concourse.bass_utils import run_bass_kernel_spmd

F32 = mybir.dt.float32
F32R = mybir.dt.float32r
BF16 = mybir.dt.bfloat16

HEADS, DH, D = 4, 32, 128
B, NCAM = 2, 6
Q = 32 * 32            # 1024 BEV queries per batch
QS = Q // 4            # 256 queries per core
KC = 28 * 60           # 1680 keys per camera
CW = 120               # key chunk width: 1680 = 14 * 120, no tail
NKCH = KC // CW        # 14
N_CORES = 8
EPS = 1e-5
SCALE = DH ** -0.5

_cached = {}


# ---------------------------------------------------------------------------
# walrus compat: this container's walrus rejects instructions carrying more
# than one semaphore wait; move excess waits onto same-engine NoOps.
_COMPUTE_ENGINES = None
_nopctr = [0]


def _split_sync_waits(nc, limit=1):
    global _COMPUTE_ENGINES
    if _COMPUTE_ENGINES is None:
        _COMPUTE_ENGINES = {
            mybir.EngineType.PE, mybir.EngineType.Activation,
            mybir.EngineType.Pool, mybir.EngineType.DVE, mybir.EngineType.SP,
        }
    for f in nc.m.functions:
        for bb in f.blocks:
            out, changed = [], False
            for inst in bb.instructions:
                si = inst.sync_info
                if (si is not None and len(si.on_wait) > limit
                        and inst.engine in _COMPUTE_ENGINES):
                    waits = list(si.on_wait)
                    n_extra = len(waits) - limit
                    for i in range(0, n_extra, limit):
                        nop = mybir.InstNoOp(name=f"wait-split-{_nopctr[0]}")
                        _nopctr[0] += 1
                        nop.engine = inst.engine
                        nop.sync_info = mybir.SyncInfo(
                            on_wait=waits[i:min(i + limit, n_extra)], on_update=[])
                        out.append(nop)
                    si.on_wait = waits[n_extra:]
                    changed = True
                out.append(inst)
            if changed:
                bb.instructions = out


# ---------------------------------------------------------------------------
def _build_program(split=True, collective=True, n_dev=N_CORES):
    nc = bass.Bass("TRN2", target_bir_lowering=False, debug=False,
                   num_devices=n_dev)

    xq = nc.dram_tensor("xq", [NCAM, D, QS], BF16, kind="ExternalInput").ap()
    xk = nc.dram_tensor("xk", [NCAM, D, KC], BF16, kind="ExternalInput").ap()
    xv = nc.dram_tensor("xv", [NCAM, D, KC], BF16, kind="ExternalInput").ap()
    # packed bf16 weights: [wq 128 | wk 128 | wv_ext 132 | w1 256 | w2 256]
    wcat = nc.dram_tensor("wcat", [D, 900], BF16, kind="ExternalInput").ap()
    wp = nc.dram_tensor("wp", [DH, HEADS, D], F32R, kind="ExternalInput").ap()
    # packed f32 per-feature consts:
    # [wbq_pairA, wbq_pairB, bp', b1_0, b1_1, b2, pre_g, pre_b, post_g,
    #  post_b] (wbq pair columns hold heads 0-1 / 2-3 in partitions 0..63)
    fcon = nc.dram_tensor("fcon", [D, 10], F32, kind="ExternalInput").ap()
    skipb = nc.dram_tensor("skipb", [D, QS], F32, kind="ExternalInput").ap()

    out = nc.dram_tensor("out", [D, QS], F32, kind="ExternalOutput").ap()

    EXP = mybir.ActivationFunctionType.Exp
    LN_ = mybir.ActivationFunctionType.Ln
    SQRT = mybir.ActivationFunctionType.Sqrt
    GELU = mybir.ActivationFunctionType.Gelu
    ADD = mybir.AluOpType.add
    MULT = mybir.AluOpType.mult

    with tile.TileContext(nc) as tc:
        with tc.tile_pool(name="consts", bufs=1) as consts, \
             tc.tile_pool(name="loads", bufs=3) as loads, \
             tc.tile_pool(name="sq", bufs=2) as sqp, \
             tc.tile_pool(name="kv", bufs=3) as kvp, \
             tc.tile_pool(name="sml", bufs=2) as sml, \
             tc.tile_pool(name="ee", bufs=3) as eep, \
             tc.tile_pool(name="fin", bufs=1) as finp:

            # ---- constants (one DMA each for the packed tensors) ----
            wcat_t = consts.tile([D, 900], BF16, name="wcat_t")
            nc.sync.dma_start(out=wcat_t, in_=wcat)
            wq_t = wcat_t[:, 0:128]
            wk_t = wcat_t[:, 128:256]
            wv_t = wcat_t[:, 256:388]          # [D, 4*33]
            w1_t = wcat_t[:, 388:644]
            w2_t = wcat_t[:, 644:900].rearrange("p (f d) -> p f d", f=2)
            wp_t = consts.tile([DH, HEADS, D], F32R, name="wp_t")
            nc.sync.dma_start(out=wp_t, in_=wp)
            fcon_t = consts.tile([D, 10], F32, name="fcon_t")
            nc.sync.dma_start(out=fcon_t, in_=fcon)
            wbq_ab = fcon_t[:, 0:2]
            bpp_t = fcon_t[:, 2:3]
            b1_t = fcon_t[:, 3:5]
            b2_t = fcon_t[:, 5:6]
            preg_t = fcon_t[:, 6:7]
            preb_t = fcon_t[:, 7:8]
            postg_t = fcon_t[:, 8:9]
            postb_t = fcon_t[:, 9:10]
            skip_t = consts.tile([D, QS], F32, name="skip_t")
            nc.sync.dma_start(out=skip_t, in_=skipb)

            eps_t = consts.tile([D, 1], F32, name="eps_t")
            nc.vector.memset(eps_t, EPS)
            onesb = consts.tile([D, 2], BF16, name="onesb")  # [1 | 1/128]
            nc.vector.memset(onesb[:, 0:1], 1.0)
            nc.vector.memset(onesb[:, 1:2], 1.0 / 128.0)
            onesr = consts.tile([D, 2], F32R, name="onesr")  # [1 | 1/128]
            nc.vector.memset(onesr[:, 0:1], 1.0)
            nc.vector.memset(onesr[:, 1:2], 1.0 / 128.0)
            ones_row = consts.tile([1, D], F32R, name="ones_row")
            nc.vector.memset(ones_row, 1.0)

            # ---- PSUM pools ----
            # banks: avt 2 + sc 2x2 + kp 1 + shared proj 1 = 8
            accpool = tc.tile_pool(name="accp", bufs=1, space="PSUM")
            accp = accpool.__enter__()
            scpool = tc.tile_pool(name="scp", bufs=2, space="PSUM")
            scp = scpool.__enter__()
            projpool = tc.tile_pool(name="projp", bufs=1, space="PSUM")
            projp = projpool.__enter__()

            avt = accp.tile([33, HEADS, QS], F32, name="avt")      # 2 banks
            # bank A: kproj [0:420] | token stats [420:476]
            ka_ps = projp.tile([D, 512], F32, name="ka_ps")        # 1 bank
            kp_ps = ka_ps[0:64, 0:420]
            st_ps = ka_ps[0:CW, 420:476].rearrange("p (j c) -> p j c", j=4)
            # bank B: vproj ping-pong [0:132|132:264] | qproj [256:512]
            sh_ps = projp.tile([D, 512], F32, name="sh_ps")        # 1 bank
            vp_ps = [sh_ps[0:CW, 0:132], sh_ps[0:CW, 132:264]]
            qp_ps = sh_ps[0:64, 256:512]

            # ---- per-camera phase 1, split so the ScalarE ops (finish)
            # can be emitted mid-attention of the previous camera and never
            # block the exp stream ----
            def load(n):
                xk_t = loads.tile([D, KC], BF16, name="xk_t", tag="xk")
                nc.sync.dma_start(out=xk_t, in_=xk[n])
                xv_t = loads.tile([D, KC], BF16, name="xv_t", tag="xv")
                nc.sync.dma_start(out=xv_t, in_=xv[n])
                xq_t = loads.tile([D, QS], BF16, name="xq_t", tag="xq")
                nc.sync.dma_start(out=xq_t, in_=xq[n])
                return xk_t, xv_t, xq_t

            def produce(n, ld):
                xk_t, xv_t, xq_t = ld

                x2k = sqp.tile([D, KC], BF16, name="x2k", tag="x2k")
                nc.vector.tensor_mul(out=x2k, in0=xk_t, in1=xk_t)
                x2v = sqp.tile([D, KC], BF16, name="x2v", tag="x2v")
                nc.vector.tensor_mul(out=x2v, in0=xv_t, in1=xv_t)
                x2q = sqp.tile([D, QS], BF16, name="x2q", tag="x2q")
                nc.vector.tensor_mul(out=x2q, in0=xq_t, in1=xq_t)

                # token-major stats via 1-col matmuls:
                # st rows: 0=k-mean, 1=v-mean, 2=k-sumsq, 3=v-sumsq
                for c in range(NKCH):
                    xkc = xk_t[:, c * CW:(c + 1) * CW]
                    xvc = xv_t[:, c * CW:(c + 1) * CW]
                    x2kc = x2k[:, c * CW:(c + 1) * CW]
                    x2vc = x2v[:, c * CW:(c + 1) * CW]
                    nc.tensor.matmul(st_ps[:, 0, c:c + 1], lhsT=xkc,
                                     rhs=onesb[:, 1:2], start=True, stop=True)
                    nc.tensor.matmul(st_ps[:, 1, c:c + 1], lhsT=xvc,
                                     rhs=onesb[:, 1:2], start=True, stop=True)
                    nc.tensor.matmul(st_ps[:, 2, c:c + 1], lhsT=x2kc,
                                     rhs=onesb[:, 0:1], start=True, stop=True)
                    nc.tensor.matmul(st_ps[:, 3, c:c + 1], lhsT=x2vc,
                                     rhs=onesb[:, 0:1], start=True, stop=True)

                st_sb = sml.tile([CW, 4, NKCH], F32, name="st_sb", tag="st")
                nc.vector.tensor_copy(out=st_sb, in_=st_ps)
                mu2 = sml.tile([CW, 2, NKCH], F32, name="mu2", tag="mu2")
                nc.vector.tensor_mul(out=mu2, in0=st_sb[:, 0:2, :],
                                     in1=st_sb[:, 0:2, :])
                var2 = sml.tile([CW, 2, NKCH], F32, name="var2", tag="var2")
                nc.vector.tensor_scalar_mul(out=var2, in0=st_sb[:, 2:4, :],
                                            scalar1=1.0 / 128.0)
                nc.vector.tensor_sub(out=var2, in0=var2, in1=mu2)

                # q stats rows (Pool C-reduce, off the PE/Act path)
                musum = sml.tile([1, QS], F32, name="musum", tag="musum")
                nc.gpsimd.tensor_reduce(out=musum, in_=xq_t,
                                        axis=mybir.AxisListType.C, op=ADD)
                sssum = sml.tile([1, QS], F32, name="sssum", tag="sssum")
                nc.gpsimd.tensor_reduce(out=sssum, in_=x2q,
                                        axis=mybir.AxisListType.C, op=ADD)
                muq = sml.tile([1, QS], F32, name="muq", tag="muq")
                nc.vector.tensor_scalar_mul(out=muq, in0=musum,
                                            scalar1=1.0 / 128.0)
                mu2q = sml.tile([1, QS], F32, name="mu2q", tag="mu2q")
                nc.vector.tensor_mul(out=mu2q, in0=muq, in1=muq)
                varq = sml.tile([1, QS], F32, name="varq", tag="varq")
                nc.vector.tensor_scalar_mul(out=varq, in0=sssum,
                                            scalar1=1.0 / 128.0)
                nc.vector.tensor_sub(out=varq, in0=varq, in1=mu2q)

                # K projection, feature-major, split in head pairs so
                # attention lhsT slices sit at base partition 0/32
                khT = [kvp.tile([64, KC], BF16, name=f"khT{p}",
                                tag=f"khT{p}") for p in range(2)]
                for j in range(4):
                    for p in range(2):
                        nc.tensor.matmul(
                            kp_ps[:, 0:420], lhsT=wk_t[:, p * 64:(p + 1) * 64],
                            rhs=xk_t[:, j * 420:(j + 1) * 420],
                            start=True, stop=True)
                        nc.vector.tensor_copy(
                            out=khT[p][:, j * 420:(j + 1) * 420],
                            in_=kp_ps[:, 0:420])

                # V projection, token-major [120, 4, 33] per chunk
                # (col 32 of each head block is 0 from wv_ext; memset to 1
                # afterwards: softmax denominator ride-along)
                vhE = kvp.tile([CW, NKCH, HEADS, 33], BF16, name="vhE",
                               tag="vhE")
                nc.gpsimd.memset(vhE[:, :, :, 32], 1.0)
                for c in range(NKCH):
                    xvc = xv_t[:, c * CW:(c + 1) * CW]
                    vp = vp_ps[c % 2]
                    nc.tensor.matmul(vp, lhsT=xvc, rhs=wv_t,
                                     start=True, stop=True)
                    nc.gpsimd.tensor_copy(
                        out=vhE[:, c, :, 0:32],
                        in_=vp.rearrange("p (h d) -> p h d", h=4)[:, :, 0:32])
                return xq_t, var2, varq, khT, vhE

            def finish(n, prod):
                xq_t, var2, varq, khT, vhE = prod
                # ln(var+eps) for both K and V in one ScalarE op
                lnb = sml.tile([CW, 2, NKCH], F32, name="lnb", tag="lnb")
                nc.scalar.activation(out=lnb, in_=var2, func=LN_,
                                     bias=eps_t[0:CW, :], scale=1.0)
                rstdk = kvp.tile([CW, NKCH], F32, name="rstdk", tag="rstdk")
                nc.scalar.activation(out=rstdk, in_=lnb[:, 0, :], func=EXP,
                                     bias=0.0, scale=-0.5)
                lnrv = kvp.tile([CW, NKCH], F32, name="lnrv", tag="lnrv")
                nc.vector.tensor_scalar_mul(out=lnrv, in0=lnb[:, 1, :],
                                            scalar1=-0.5)

                sdq = sml.tile([1, QS], F32, name="sdq", tag="sdq")
                nc.scalar.activation(out=sdq, in_=varq, func=SQRT,
                                     bias=eps_t[0:1, :], scale=1.0)
                rqrow = sml.tile([1, QS], F32, name="rqrow", tag="rqrow")
                nc.vector.reciprocal(out=rqrow, in_=sdq)
                rqbc = sml.tile([64, QS], F32, name="rqbc", tag="rqbc")
                nc.gpsimd.partition_broadcast(rqbc, rqrow, channels=64)
                qhT = [kvp.tile([64, QS], BF16, name=f"qhT{p}",
                                tag=f"qhT{p}") for p in range(2)]
                for p in range(2):
                    nc.tensor.matmul(qp_ps,
                                     lhsT=wq_t[:, p * 64:(p + 1) * 64],
                                     rhs=xq_t, start=True, stop=True)
                    nc.vector.tensor_mul(out=qhT[p], in0=qp_ps, in1=rqbc)
                    nc.vector.tensor_scalar_add(
                        out=qhT[p], in0=qhT[p],
                        scalar1=wbq_ab[0:64, p:p + 1])
                return khT, vhE, rstdk, lnrv, qhT

            # ---- attention for one camera (chunk range) ----
            def attention(n, cam, c0, c1):
                khT, vhE, rstdk, lnrv, qhT = cam
                for c in range(c0, c1):
                    sc_ps = scp.tile([CW, HEADS, QS], F32, name="sc_ps",
                                     tag="sc")
                    for h in range(HEADS):
                        p, hh = divmod(h, 2)
                        nc.tensor.matmul(
                            sc_ps[:, h, :],
                            lhsT=khT[p][hh * DH:(hh + 1) * DH,
                                        c * CW:(c + 1) * CW],
                            rhs=qhT[p][hh * DH:(hh + 1) * DH, :],
                            start=True, stop=True)
                    et = eep.tile([CW, HEADS, QS], BF16, name="et", tag="et")
                    nc.scalar.activation(out=et, in_=sc_ps, func=EXP,
                                         bias=lnrv[:, c:c + 1],
                                         scale=rstdk[:, c:c + 1])
                    first = (n == 0 and c == 0)
                    last = (n == NCAM - 1 and c == NKCH - 1)
                    for h in range(HEADS):
                        nc.tensor.matmul(
                            avt[:, h, :],
                            lhsT=vhE[:, c, h, 0:33],
                            rhs=et[:, h, :],
                            start=first, stop=last)

            # ---- pipelined schedule: produce(n+1) and finish(n+1) are
            # emitted around the first half of attention(n) so no engine's
            # in-order queue ever blocks the exp stream ----
            cams = [None] * NCAM
            prods = [None] * NCAM
            ld0 = load(0)
            ld1 = load(1)
            prods[0] = produce(0, ld0)
            cams[0] = finish(0, prods[0])
            prods[1] = produce(1, ld1)
            for n in range(NCAM):
                attention(n, cams[n], 0, 4)
                if n + 1 < NCAM:
                    cams[n + 1] = finish(n + 1, prods[n + 1])
                attention(n, cams[n], 4, 9)
                if n + 2 < NCAM:
                    prods[n + 2] = produce(n + 2, load(n + 2))
                attention(n, cams[n], 9, NKCH)

            # ---- tail: normalize, project, skip, LN, MLP, LN ----
            # per-(head,query) denominator: reciprocal straight from PSUM,
            # broadcast via PE matmul (ones row), normalize from PSUM
            rden = finp.tile([1, HEADS, QS], F32R, name="rden")
            with nc.allow_low_precision(reason="f32r denominator"):
                nc.vector.reciprocal(out=rden, in_=avt[32:33, :, :])

            projpool.__exit__(None, None, None)
            scpool.__exit__(None, None, None)
            fpool = tc.tile_pool(name="fps", bufs=1, space="PSUM")
            fps = fpool.__enter__()
            rd_ps = fps.tile([DH, 512], F32, name="rd_ps")
            anorm = finp.tile([DH, HEADS, QS], F32R, name="anorm")
            rden_f = rden.rearrange("p h q -> p (h q)")
            anorm_f = anorm.rearrange("p h q -> p (h q)")
            avt_f = avt[0:32, :, :].rearrange("p h q -> p (h q)")
            for j in range(2):
                nc.tensor.matmul(rd_ps, lhsT=ones_row[:, 0:DH],
                                 rhs=rden_f[:, j * 512:(j + 1) * 512],
                                 start=True, stop=True)
                nc.vector.tensor_mul(out=anorm_f[:, j * 512:(j + 1) * 512],
                                     in0=avt_f[:, j * 512:(j + 1) * 512],
                                     in1=rd_ps)

            zo_ps = fps.tile([D, QS], F32, name="zo_ps", tag="zo")
            for h in range(HEADS):
                nc.tensor.matmul(zo_ps, lhsT=wp_t[:, h, :],
                                 rhs=anorm[:, h, :],
                                 start=(h == 0), stop=(h == HEADS - 1))
            zt = finp.tile([D, QS], F32R, name="zt")
            nc.vector.tensor_add(out=zt, in0=zo_ps, in1=skip_t)
            nc.vector.tensor_scalar_add(out=zt, in0=zt, scalar1=bpp_t)

            row_ps = fps.tile([1, 2, QS], F32, name="row_ps")
            mr_ps = fps.tile([D, 2, QS], F32, name="mr_ps")

            def feat_ln(src, gain, bias_, dst_dt, nm):
                """LayerNorm across partitions (d) of src [128, QS]."""
                s2 = finp.tile([D, QS], F32R, name=nm + "_s2", tag="ln_s2")
                nc.vector.tensor_mul(out=s2, in0=src, in1=src)
                nc.tensor.matmul(row_ps[:, 0, :], lhsT=onesr[:, 1:2],
                                 rhs=src, start=True, stop=True)
                nc.tensor.matmul(row_ps[:, 1, :], lhsT=onesr[:, 0:1],
                                 rhs=s2, start=True, stop=True)
                murow = sml.tile([1, QS], F32R, name=nm + "_mu", tag="ln_mu")
                nc.vector.tensor_copy(out=murow, in_=row_ps[:, 0, :])
                m2 = sml.tile([1, QS], F32, name=nm + "_m2", tag="ln_m2")
                nc.vector.tensor_mul(out=m2, in0=murow, in1=murow)
                vr = sml.tile([1, QS], F32, name=nm + "_vr", tag="ln_vr")
                nc.vector.tensor_scalar_mul(out=vr, in0=row_ps[:, 1, :],
                                            scalar1=1.0 / 128.0)
                nc.vector.tensor_sub(out=vr, in0=vr, in1=m2)
                sd = sml.tile([1, QS], F32, name=nm + "_sd", tag="ln_sd")
                nc.scalar.activation(out=sd, in_=vr, func=SQRT,
                                     bias=eps_t[0:1, :], scale=1.0)
                rs = sml.tile([1, QS], F32R, name=nm + "_rs", tag="ln_rs")
                with nc.allow_low_precision(reason="f32r rstd"):
                    nc.vector.reciprocal(out=rs, in_=sd)
                nc.tensor.matmul(mr_ps[:, 0, :], lhsT=ones_row,
                                 rhs=murow, start=True, stop=True)
                nc.tensor.matmul(mr_ps[:, 1, :], lhsT=ones_row,
                                 rhs=rs, start=True, stop=True)
                zc = finp.tile([D, QS], F32R, name=nm + "_zc", tag="ln_zc")
                nc.vector.tensor_sub(out=zc, in0=src, in1=mr_ps[:, 0, :])
                zm = finp.tile([D, QS], F32R, name=nm + "_zm", tag="ln_zm")
                nc.vector.tensor_mul(out=zm, in0=zc, in1=mr_ps[:, 1, :])
                dst = finp.tile([D, QS], dst_dt, name=nm, tag="ln_dst")
                nc.vector.tensor_scalar(out=dst, in0=zm, scalar1=gain,
                                        scalar2=bias_, op0=MULT, op1=ADD)
                return dst

            zhat = feat_ln(zt, preg_t, preb_t, BF16, "zhat")

            h_ps = fps.tile([D, 2, QS], F32, name="h_ps")
            for f in range(2):
                nc.tensor.matmul(h_ps[:, f, :],
                                 lhsT=w1_t[:, f * D:(f + 1) * D],
                                 rhs=zhat, start=True, stop=True)
            gel = finp.tile([D, 2, QS], BF16, name="gel")
            for f in range(2):
                nc.scalar.activation(out=gel[:, f, :], in_=h_ps[:, f, :],
                                     func=GELU, bias=b1_t[:, f:f + 1],
                                     scale=1.0)
            o2_ps = fps.tile([D, QS], F32, name="o2_ps", tag="zo")
            for f in range(2):
                nc.tensor.matmul(o2_ps, lhsT=w2_t[:, f, :], rhs=gel[:, f, :],
                                 start=(f == 0), stop=(f == 1))
            res = finp.tile([D, QS], F32R, name="res")
            nc.vector.tensor_scalar_add(out=res, in0=o2_ps, scalar1=b2_t)
            nc.vector.tensor_add(out=res, in0=res, in1=zhat)

            final = feat_ln(res, postg_t, postb_t, F32, "final")
            nc.sync.dma_start(out=out, in_=final)
            fpool.__exit__(None, None, None)
            accpool.__exit__(None, None, None)

    if split:
        _split_sync_waits(nc)
    return nc


# ---------------------------------------------------------------------------
def _prep_core_inputs(b, r, q, k, v, skip, q_ln_g, q_ln_b, Wq, bq, k_ln_g,
                      k_ln_b, Wk, bk, v_ln_g, v_ln_b, Wv, bv, Wp, bp,
                      pre_g, pre_b, W1, b1, W2, b2, post_g, post_b):
    f32 = np.float32
    bf16 = ml_dtypes.bfloat16

    def fold(W, g):
        wg = g[:, None] * W
        return (wg - wg.sum(0, keepdims=True) / 128.0).astype(f32)

    wq_all = SCALE * fold(Wq, q_ln_g)                      # [D, 128]
    wk_all = fold(Wk, k_ln_g)                              # [D, 128]
    wv_f = fold(Wv, v_ln_g)                                # [D, 128]
    wv_ext = np.zeros((D, HEADS, 33), f32)
    wv_ext[:, :, 0:32] = wv_f.reshape(D, HEADS, DH)
    wcat = np.concatenate([
        wq_all, wk_all, wv_ext.reshape(D, HEADS * 33),
        W1.astype(f32),
        W2.reshape(2, D, D).transpose(1, 0, 2).reshape(D, 2 * D),
    ], axis=1).astype(bf16)

    # bias folding: q_ln_b -> wbq (added to q-heads); v_ln_b -> bp'
    # (rides through attention as a constant, then Wp); k_ln_b drops out
    # (adds a per-query constant to all logits -> softmax invariant).
    wbq = (SCALE * (Wq.T @ q_ln_b)).astype(f32)            # [128]
    wbv = Wv.T @ v_ln_b                                    # [128]
    bpp = bp + Wp.T @ wbv                                  # [D]
    pad = np.zeros(64, f32)
    fcon = np.stack([
        np.concatenate([wbq[0:64], pad]),
        np.concatenate([wbq[64:128], pad]),
        bpp, b1[0:D], b1[D:2 * D], b2,
        pre_g, pre_b, post_g, post_b,
    ], axis=1).astype(f32)

    sl = slice(r * QS, (r + 1) * QS)
    return {
        "xq": np.ascontiguousarray(
            q[b].reshape(NCAM, D, Q)[:, :, sl]).astype(bf16),
        "xk": np.ascontiguousarray(k[b].reshape(NCAM, D, KC)).astype(bf16),
        "xv": np.ascontiguousarray(v[b].reshape(NCAM, D, KC)).astype(bf16),
        "wcat": wcat,
        "wp": np.ascontiguousarray(
            Wp.reshape(HEADS, DH, D).transpose(1, 0, 2), f32),
        "fcon": fcon,
        "skipb": np.ascontiguousarray(skip[b].reshape(D, Q)[:, sl], f32),
    }


def kernel(**inputs):
    if "nc" not in _cached:
        _cached["nc"] = _build_program()
    nc = _cached["nc"]
    args = {kk: np.asarray(vv) for kk, vv in inputs.items()}
    in_maps = [_prep_core_inputs(c // 4, c % 4, **args) for c in range(N_CORES)]
    res = run_bass_kernel_spmd(nc, in_maps, core_ids=list(range(N_CORES)))
    full = np.zeros((B, D, Q), np.float32)
    for c in range(N_CORES):
        b, r = c // 4, c % 4
        full[b][:, r * QS:(r + 1) * QS] = res.results[c]["out"]
    return full.reshape(B, D, 32, 32)
